# revision 1
# baseline (speedup 1.0000x reference)
"""nn_Attention Trainium2 Bass kernel (v2 — interleaved pipeline).

Full attention forward: x->(q,k,v) with l2-normalized weights, per-head-dim
l2 norm + learned qk scale, interleaved RoPE, causal SDPA, output projection
with column-l2-normalized wo.

Sharding: TP=4 over heads (8 heads/core) x DP=2 over batch across 8 cores.
Each core computes a partial [2048, 2048] output for its batch; host sums
the 4 TP partials per batch.

v2 changes vs v1:
- single interleaved loop per 512-row block: proj -> attention -> yproj,
  so DVE rope work, Act exp work and PE matmuls overlap across phases.
- q/k transposes via DMA xbar (dma_start_transpose) instead of PE
  transposes + DVE copies.
- causal mask as a single 128x128 triangle multiply on the Pool engine.
- lg/pv matmuls trimmed to the live columns on diagonal blocks.
- softmax denominators: v's 65th ones-column -> psum row 64 -> stashed ->
  gathered by DMA -> PE-transposed to si-partition layout -> one cheap
  [128,32] reciprocal -> transposed back -> rank-8 indicator matmul
  broadcast (replaces 3.3us-per-call wide DVE reciprocals).
- x streamed per 512-column block (2-deep) instead of fully resident.
- yproj results DMA'd directly from PSUM to DRAM.
"""
import sys
import os
import math
from contextlib import ExitStack

sys.path.insert(0, "/opt/trn_rl_repo")

import numpy as np
import ml_dtypes

BF16 = ml_dtypes.bfloat16

B, S, DIM = 2, 2048, 2048
HEADS, DH = 32, 64
THETA = 10000.0
NCORES = 8
TP = 4             # head-parallel ways
HPC = HEADS // TP  # heads per core = 8
E = HPC * DH       # per-core qkv width = 512
ET = E // 128      # e-tiles per core = 4
DT = DIM // 128    # contraction d-tiles = 16
SB = S // 512      # 512-wide seq blocks = 4
SS = S // 128      # 128-wide seq blocks = 16

_CACHE = {}


def _l2n(w, axis):
    n = np.sqrt((w.astype(np.float64) ** 2).sum(axis=axis, keepdims=True))
    n = np.maximum(n, 1e-12)
    return (w / n).astype(np.float32)


def _build_program():
    import concourse.bass as bass
    from concourse import bacc
    import concourse.mybir as mybir
    import concourse.tile as tile
    from concourse.masks import make_identity

    f32 = mybir.dt.float32
    bf16 = mybir.dt.bfloat16
    AF = mybir.ActivationFunctionType
    AX = mybir.AxisListType
    OP = mybir.AluOpType

    nc = bacc.Bacc("TRN2", target_bir_lowering=False)

    xT = nc.dram_tensor("xT", [DIM, S], bf16, kind="ExternalInput")
    wqT = nc.dram_tensor("wqT", [DIM, E], bf16, kind="ExternalInput")
    wkT = nc.dram_tensor("wkT", [DIM, E], bf16, kind="ExternalInput")
    wvT = nc.dram_tensor("wvT", [DIM, E], bf16, kind="ExternalInput")
    woT = nc.dram_tensor("woT", [E, DIM], bf16, kind="ExternalInput")
    cosd = nc.dram_tensor("cosd", [128, SS * DH], bf16, kind="ExternalInput")
    sind = nc.dram_tensor("sind", [128, SS * DH], bf16, kind="ExternalInput")
    trid = nc.dram_tensor("trid", [128, 128], bf16, kind="ExternalInput")
    ind8d = nc.dram_tensor("ind8d", [8, 512], bf16, kind="ExternalInput")
    Y = nc.dram_tensor("Y", [S, DIM], f32, kind="ExternalOutput")

    with tile.TileContext(nc) as tc, ExitStack() as ctx:
        const = ctx.enter_context(tc.tile_pool(name="const", bufs=1))
        wpool = ctx.enter_context(tc.tile_pool(name="wpool", bufs=4))
        xpool = ctx.enter_context(tc.tile_pool(name="xpool", bufs=2))
        qkv = ctx.enter_context(tc.tile_pool(name="qkv", bufs=1))
        work = ctx.enter_context(tc.tile_pool(name="work", bufs=1))
        expool = ctx.enter_context(tc.tile_pool(name="expool", bufs=4))
        psA = ctx.enter_context(
            tc.tile_pool(name="psA", bufs=4, space="PSUM"))
        psL = ctx.enter_context(
            tc.tile_pool(name="psL", bufs=2, space="PSUM"))

        # --- weights (wq first, quartered, so proj can start early) ---
        wq_sb = [wpool.tile([128, 4, E], bf16, tag=f"wq{j}", bufs=1, name=f"wq{j}")
                 for j in range(4)]
        wk_sb = wpool.tile([128, DT, E], bf16, tag="wk", bufs=1)
        wv_sb = wpool.tile([128, DT, E], bf16, tag="wv", bufs=1)
        wo_sb = wpool.tile([128, ET, DIM], bf16, tag="wo", bufs=1)
        wqr = wqT.rearrange("(t p) e -> p t e", p=128)

        xtiles = {}

        def load_x(st):
            ts = [xpool.tile([128, 4, 512], bf16, tag=f"x{j}", bufs=2,
                             name=f"xst{st}_{j}") for j in range(4)]
            src = xT[:, st * 512:(st + 1) * 512].rearrange(
                "(t p) s -> p t s", p=128)
            for j in range(4):
                nc.sync.dma_start(ts[j], src[:, j * 4:(j + 1) * 4, :])
            return ts

        # interleave wq quarters with x quarters so dt=0..3 can start early
        x0src = xT[:, 0:512].rearrange("(t p) s -> p t s", p=128)
        x0 = [xpool.tile([128, 4, 512], bf16, tag=f"x{j}", bufs=2,
                         name=f"xst0_{j}") for j in range(4)]
        for j in range(4):
            nc.sync.dma_start(wq_sb[j], wqr[:, j * 4:(j + 1) * 4, :])
            nc.sync.dma_start(x0[j], x0src[:, j * 4:(j + 1) * 4, :])
        xtiles[0] = x0
        nc.sync.dma_start(wk_sb, wkT.rearrange("(t p) e -> p t e", p=128))
        nc.sync.dma_start(wv_sb, wvT.rearrange("(t p) e -> p t e", p=128))

        # --- constants ---
        cos_sb = const.tile([128, SS, DH], bf16)
        sin_sb = const.tile([128, SS, DH], bf16)
        nc.sync.dma_start(cos_sb, cosd.rearrange("p (b d) -> p b d", d=DH))
        nc.sync.dma_start(sin_sb, sind.rearrange("p (b d) -> p b d", d=DH))
        tri = const.tile([128, 128], bf16)
        nc.sync.dma_start(tri, trid[:, :])
        ind8 = const.tile([8, 512], bf16)
        nc.sync.dma_start(ind8, ind8d[:, :])
        nc.sync.dma_start(wo_sb, woT.rearrange("(t p) e -> p t e", p=128))
        identf = const.tile([128, 128], f32)
        make_identity(nc, identf)
        ident = const.tile([128, 128], bf16)
        make_identity(nc, ident)

        # --- persistent activations ---
        qTall = qkv.tile([128, ET, S], bf16, tag="qT")
        kTall = qkv.tile([128, ET, S], bf16, tag="kT")
        v_sb = qkv.tile([128, SS, HPC, 66], bf16, tag="v")
        stash = qkv.tile([65, HPC, 512], bf16, tag="stash")
        nc.vector.memset(v_sb[:, :, :, 64:66], 1.0)

        def norm_rope(ps, dstT, st, su):
            """psum [si,e] natural -> per-head l2norm, rope, bf16,
            -> DMA-transpose into dstT columns."""
            sblk = st * 4 + su
            sq = work.tile([128, E], bf16, tag="sq", bufs=2)
            nc.scalar.square(sq, ps)
            ssq = work.tile([128, HPC], f32, tag="ssq", bufs=2)
            nc.vector.tensor_reduce(
                ssq, sq.rearrange("p (h d) -> p h d", d=DH),
                axis=AX.X, op=OP.add)
            # rsqrt via magic-number seed + 2 Newton iterations (DVE only;
            # keeps the Act engine free of sqrt so its activation table
            # never leaves the exp set)
            inv = work.tile([128, HPC], f32, tag="inv", bufs=2)
            ssq_i = ssq.bitcast(mybir.dt.int32)
            inv_i = inv.bitcast(mybir.dt.int32)
            nc.vector.tensor_scalar(inv_i, ssq_i, 1, None,
                                    op0=OP.arith_shift_right)
            nc.vector.tensor_scalar(inv_i, inv_i, 0x5f3759df, -1,
                                    op0=OP.subtract, op1=OP.mult)
            y2 = work.tile([128, HPC], f32, tag="y2", bufs=2)
            for _ in range(2):
                nc.vector.tensor_mul(y2, inv, inv)
                nc.vector.scalar_tensor_tensor(
                    y2, ssq, -0.5, y2, op0=OP.mult, op1=OP.mult)
                nc.vector.tensor_scalar(y2, y2, 1.5, None, op0=OP.add)
                nc.vector.tensor_mul(inv, inv, y2)
            qn = work.tile([128, HPC, DH], bf16, tag="qn", bufs=2)
            nc.vector.tensor_mul(
                qn, ps.rearrange("p (h d) -> p h d", d=DH),
                inv.unsqueeze(2).broadcast_to([128, HPC, DH]))
            cosb = cos_sb[:, sblk:sblk + 1, :].broadcast_to([128, HPC, DH])
            sinb = sin_sb[:, sblk:sblk + 1, :].broadcast_to([128, HPC, DH])
            rot = work.tile([128, HPC, 2, 32], bf16, tag="rot", bufs=2)
            qn4 = qn.rearrange("p h (t u) -> p h t u", u=32)
            nc.vector.tensor_copy(rot[:, :, 0:1, :], qn4[:, :, 1:2, :])
            nc.vector.tensor_copy(rot[:, :, 1:2, :], qn4[:, :, 0:1, :])
            nc.vector.tensor_mul(rot.rearrange("p h t u -> p h (t u)"),
                                 rot.rearrange("p h t u -> p h (t u)"), sinb)
            nc.vector.tensor_mul(qn, qn, cosb)
            qo = work.tile([128, E], bf16, tag="qo", bufs=2)
            nc.vector.tensor_add(
                qo, qn.rearrange("p h d -> p (h d)"),
                rot.rearrange("p h t u -> p (h t u)"))
            nc.sync.dma_start_transpose(
                dstT[:, :, sblk * 128:(sblk + 1) * 128], qo)

        def proj_half_gen(w_sb, kind, st, s0):
            """Emits one su-pair of a proj wave in 4 chunks of 8 matmuls
            (yields between chunks so attention can interleave)."""
            xt = xtiles[st]
            quartered = isinstance(w_sb, list)
            prs = [psA.tile([128, E], f32, tag="ps",
                            name=f"p{kind}{st}_{s0 + j}")
                   for j in range(2)]
            for dtc in range(4):
                for dt in range(dtc * 4, dtc * 4 + 4):
                    if quartered:
                        wslice = w_sb[dt // 4][:, dt % 4, :]
                    else:
                        wslice = w_sb[:, dt, :]
                    for j in range(2):
                        su = s0 + j
                        nc.tensor.matmul(
                            prs[j],
                            xt[dt // 4][:, dt % 4,
                                        su * 128:(su + 1) * 128],
                            wslice,
                            start=(dt == 0), stop=(dt == DT - 1))
                if dtc < 3:
                    yield
            for j in range(2):
                su = s0 + j
                if kind == "v":
                    nc.vector.tensor_copy(
                        v_sb[:, st * 4 + su, :, 0:64],
                        prs[j].rearrange("p (h d) -> p h d", d=DH))
                else:
                    norm_rope(prs[j], qTall if kind == "q" else kTall,
                              st, su)

        def proj_half(w_sb, kind, st, s0):
            for _ in proj_half_gen(w_sb, kind, st, s0):
                pass

        def proj_all(st):
            for w_sb, kind in ((wq_sb, "q"), (wk_sb, "k"), (wv_sb, "v")):
                for s0 in (0, 2):
                    proj_half(w_sb, kind, st, s0)

        def attn_pair(i, ha, hb, feed=None):
            """Head-paired attention: heads (h, h+2) share PE tile config
            (same hp), so lg and pv matmuls run in same-config groups of 4
            with alternating PSUM banks."""
            last = 4 * i + 3
            npr = 2 * (i + 1)
            if True:
                hp = (ha % 2) * 64
                ets = {ha: ha // 2, hb: hb // 2}
                pvs = {h: psA.tile([128, 512], f32, tag="ps",
                                   name=f"pv{i}_{h}")
                       for h in (ha, hb)}
                lgs = {}

                def lg4(p):
                    for h in (ha, hb):
                        lgs[(h, p)] = psL.tile(
                            [128, 2, 512], f32, tag="lg",
                            name=f"lg{i}_{h}_{p}")
                    for b in range(2):
                        sjb = 2 * p + b
                        r = sjb - 4 * i
                        c0 = r * 128 if r > 0 else 0
                        for h in (ha, hb):
                            nc.tensor.matmul(
                                lgs[(h, p)][:, b, c0:],
                                kTall[hp:hp + 64, ets[h],
                                      sjb * 128:(sjb + 1) * 128],
                                qTall[hp:hp + 64, ets[h],
                                      i * 512 + c0:(i + 1) * 512],
                                start=True, stop=True)

                lg4(0)
                for p in range(npr):
                    exs = {}
                    for h in (ha, hb):
                        lg2 = lgs.pop((h, p))
                        ex = expool.tile([128, 2, 512], bf16, tag="ex",
                                         name=f"ex{i}_{h}_{p}")
                        if 2 * p - 4 * i >= 0:  # diagonal pair: match trim
                            for b in range(2):
                                c0 = max(0, (2 * p + b - 4 * i)) * 128
                                nc.scalar.activation(ex[:, b, c0:],
                                                     lg2[:, b, c0:], AF.Exp)
                        else:
                            nc.scalar.activation(ex, lg2, AF.Exp)
                        exs[h] = ex
                    if p + 1 < npr:
                        lg4(p + 1)
                    for b in range(2):
                        sjb = 2 * p + b
                        r = sjb - 4 * i
                        c0 = r * 128 if r > 0 else 0
                        if r >= 0:
                            for h in (ha, hb):
                                nc.gpsimd.tensor_mul(
                                    exs[h][:, b, r * 128:(r + 1) * 128],
                                    exs[h][:, b, r * 128:(r + 1) * 128],
                                    tri)
                        for h in (ha, hb):
                            nc.tensor.matmul(
                                pvs[h][0:66, c0:],
                                v_sb[:, sjb, h, :],
                                exs[h][:, b, c0:],
                                start=(sjb == 0), stop=(sjb == last))
                    if feed is not None:
                        feed()
                for h in (ha, hb):
                    nc.vector.tensor_copy(stash[:, h, :], pvs[h][0:65, :])

        def normalize_gather(i):
            den = work.tile([8, 512], bf16, tag="den", bufs=2)
            nc.scalar.dma_start(den, stash[64:65, :, :])
            return den

        def normalize_recip(i, den):
            """den rows -> si-partition layout via PE transposes -> one
            cheap [128,32] DVE reciprocal."""
            invT = psA.tile([128, 32], bf16, tag="ps")
            for c in range(4):
                nc.tensor.transpose(
                    invT[:, c * 8:(c + 1) * 8],
                    den[:, c * 128:(c + 1) * 128], ident[0:8, 0:8])
            inv_sb = work.tile([128, 32], f32, tag="invsb", bufs=2)
            nc.vector.reciprocal(inv_sb, invT)
            return inv_sb

        def normalize_apply(i, inv_sb):
            """transpose back to row layout, rank-8 indicator broadcast,
            per-head mul into qTall."""
            invrow = psA.tile([8, 4, 128], f32, tag="ps")
            for c in range(4):
                nc.tensor.transpose(
                    invrow[:, c, :], inv_sb[:, c * 8:(c + 1) * 8], identf)
            inv_row = work.tile([8, 512], bf16, tag="invrowsb", bufs=2)
            nc.vector.tensor_copy(
                inv_row, invrow.rearrange("p c j -> p (c j)"))
            for h in range(HPC):
                et, hp = h // 2, (h % 2) * 64
                bc = psA.tile([64, 512], f32, tag="ps", name=f"bc{i}_{h}")
                nc.tensor.matmul(bc, ind8[:, h * 64:(h + 1) * 64], inv_row,
                                 start=True, stop=True)
                nc.vector.tensor_mul(
                    qTall[hp:hp + 64, et, i * 512:(i + 1) * 512],
                    stash[0:64, h, :], bc)

        def yproj_quarter_gen(ib):
            if True:
                for nd0 in (0, 2):
                    pss = [psA.tile([128, 512], f32, tag="ps",
                                    name=f"y{ib}_{nd0 + j}")
                           for j in range(2)]
                    for ket in range(ET):
                        for j in range(2):
                            nd = nd0 + j
                            nc.tensor.matmul(
                                pss[j],
                                qTall[:, ket, ib * 128:(ib + 1) * 128],
                                wo_sb[:, ket, nd * 512:(nd + 1) * 512],
                                start=(ket == 0), stop=(ket == ET - 1))
                    for j in range(2):
                        nd = nd0 + j
                        ys = work.tile([128, 512], f32, tag="ys", bufs=2)
                        if nd % 2 == 0:
                            nc.vector.tensor_copy(ys, pss[j])
                        else:
                            nc.scalar.copy(ys, pss[j])
                        nc.sync.dma_start(
                            Y[ib * 128:(ib + 1) * 128,
                              nd * 512:(nd + 1) * 512], ys)
                    if nd0 == 0:
                        yield

        def yproj_quarter(ib):
            for _ in yproj_quarter_gen(ib):
                pass

        def yproj_block(i):
            for ib in range(4 * i, 4 * i + 4):
                yproj_quarter(ib)

        xtiles[1] = load_x(1)
        proj_all(0)
        yq_backlog = []
        pairs = ((0, 2), (4, 6), (1, 3), (5, 7))
        halves = ((wq_sb, "q", 0), (wq_sb, "q", 2), (wk_sb, "k", 0),
                  (wk_sb, "k", 2), (wv_sb, "v", 0), (wv_sb, "v", 2))
        for st in range(SB):
            nxt = st + 1 < SB
            if st + 2 < SB:
                xtiles[st + 2] = load_x(st + 2)
            gens = []
            if nxt:
                gens += [proj_half_gen(w, k, st + 1, s)
                         for (w, k, s) in halves]
            if st > 0:
                yq_backlog.extend(range(4 * (st - 1), 4 * st))
            ntake = 2 if nxt else len(yq_backlog)
            for _ in range(min(ntake, len(yq_backlog))):
                gens.append(yproj_quarter_gen(yq_backlog.pop(0)))
            gq = list(gens)

            def feed():
                while gq:
                    try:
                        next(gq[0])
                        return
                    except StopIteration:
                        gq.pop(0)

            for ha, hb in pairs:
                attn_pair(st, ha, hb, feed)
            den = normalize_gather(st)
            inv_sb = normalize_recip(st, den)
            while gq:
                feed()
            normalize_apply(st, inv_sb)
        yproj_block(SB - 1)

    return nc


def _host_prep(x, wq, wk, wv, wo, qk_scale):
    """Returns per-core input dicts."""
    perm = np.concatenate([np.arange(0, DH, 2), np.arange(1, DH, 2)])
    wq_n = _l2n(wq, -1).reshape(HEADS, DH, DIM)[:, perm, :].reshape(HEADS * DH, DIM)
    wk_n = _l2n(wk, -1).reshape(HEADS, DH, DIM)[:, perm, :].reshape(HEADS * DH, DIM)
    wv_n = _l2n(wv, -1)
    wo_n = _l2n(wo, 0)
    sp = qk_scale.astype(np.float64)[perm]

    # rope tables with qk_scale folded in; permuted-block layout
    half = np.arange(0, DH, 2)
    freqs = 1.0 / (THETA ** (half.astype(np.float64) / DH))      # (32,)
    ang = np.arange(S, dtype=np.float64)[:, None] * freqs[None]  # (S, 32)
    cos_h, sin_h = np.cos(ang), np.sin(ang)
    cos_p = np.concatenate([cos_h, cos_h], 1)                    # (S, 64)
    sin_e = np.concatenate([-sin_h, sin_h], 1)
    cos_eff = (cos_p * sp[None, :]).astype(np.float32)
    swap_sp = np.concatenate([sp[32:], sp[:32]])
    sin_eff = (sin_e * swap_sp[None, :]).astype(np.float32)
    # device layout [128, SS*DH]: [p, b*64+c] = tbl[b*128+p, c]
    cosd = np.ascontiguousarray(
        cos_eff.reshape(SS, 128, DH).transpose(1, 0, 2).reshape(128, SS * DH))
    sind = np.ascontiguousarray(
        sin_eff.reshape(SS, 128, DH).transpose(1, 0, 2).reshape(128, SS * DH))

    # causal triangle for the diagonal 128-blocks: keep sjl <= sil
    sjl = np.arange(128)[:, None]
    sil = np.arange(128)[None, :]
    trid = (sjl <= sil).astype(np.float32)

    # indicator for denominator broadcast: ind8[k, h*64+m] = (k == h)
    ind8 = np.zeros((8, 512), dtype=np.float32)
    for h in range(8):
        ind8[h, h * 64:(h + 1) * 64] = 1.0

    in_maps = []
    for c in range(NCORES):
        b, t = divmod(c, TP)
        e0 = t * E
        in_maps.append({
            "xT": np.ascontiguousarray(x[b].T).astype(BF16),
            "wqT": np.ascontiguousarray(wq_n[e0:e0 + E].T).astype(BF16),
            "wkT": np.ascontiguousarray(wk_n[e0:e0 + E].T).astype(BF16),
            "wvT": np.ascontiguousarray(wv_n[e0:e0 + E].T).astype(BF16),
            "woT": np.ascontiguousarray(wo_n[:, e0:e0 + E].T).astype(BF16),
            "cosd": cosd.astype(BF16), "sind": sind.astype(BF16),
            "trid": trid.astype(BF16), "ind8d": ind8.astype(BF16),
        })
    return in_maps


def _install_profile_hook():
    """antenv.axon_hooks is absent in this image; shim it and register the
    ctypes NTFF hook against /opt/axon/libaxon_pjrt.so (mirrors trn_boot)."""
    import types
    import ctypes
    import contextlib

    try:
        from antenv.axon_hooks import get_axon_ntff_profile_hook  # noqa
        return
    except ImportError:
        pass
    import antenv
    mod = types.ModuleType("antenv.axon_hooks")
    state = {}
    mod.set_axon_ntff_profile_hook = lambda h: state.__setitem__("h", h)
    mod.get_axon_ntff_profile_hook = lambda: state.get("h")
    sys.modules["antenv.axon_hooks"] = mod
    antenv.axon_hooks = mod

    so_path = "/opt/axon/libaxon_pjrt.so"
    lib = ctypes.CDLL(so_path)
    if not hasattr(lib, "axon_start_nrt_profile"):
        return
    lib.axon_start_nrt_profile.argtypes = [
        ctypes.POINTER(ctypes.c_int64), ctypes.c_size_t]
    lib.axon_start_nrt_profile.restype = ctypes.c_int64
    lib.axon_stop_nrt_profile.argtypes = [ctypes.c_char_p]
    lib.axon_stop_nrt_profile.restype = ctypes.c_int64

    @contextlib.contextmanager
    def _hook(output_dir, device_ids):
        import jax
        jax.devices()
        if device_ids:
            ids = (ctypes.c_int64 * len(device_ids))(*device_ids)
            rc = lib.axon_start_nrt_profile(ids, len(device_ids))
        else:
            rc = lib.axon_start_nrt_profile(None, 0)
        if rc != 0:
            raise RuntimeError(f"axon_start_nrt_profile rc={rc}")
        try:
            yield
        finally:
            n = lib.axon_stop_nrt_profile(str(output_dir).encode())
            print(f"profile: {n} file(s) written to {output_dir}",
                  file=sys.stderr)

    mod.set_axon_ntff_profile_hook(_hook)


def kernel(x, wq, wk, wv, wo, qk_scale, _profile=False):
    from concourse.bass_utils import run_bass_kernel_spmd

    if _profile:
        _install_profile_hook()

    if "nc" not in _CACHE:
        nc = _build_program()
        nc.finalize()
        _CACHE["nc"] = nc
    nc = _CACHE["nc"]
    in_maps = _host_prep(np.asarray(x), np.asarray(wq), np.asarray(wk),
                         np.asarray(wv), np.asarray(wo), np.asarray(qk_scale))
    res = run_bass_kernel_spmd(nc, in_maps, core_ids=list(range(NCORES)),
                               trace=_profile)
    outs = res.results
    y = np.empty((B, S, DIM), dtype=np.float32)
    for b in range(B):
        y[b] = sum(outs[b * TP + t]["Y"] for t in range(TP))
    if _profile:
        _CACHE["last_exec_time_ns"] = res.exec_time_ns
        _CACHE["last_profile"] = res.profile_json
    return y



# revision 7
# speedup vs baseline: 1.0514x; 1.0514x over previous
"""nn_Attention Trainium2 Bass kernel (v3 — stall-free scheduling).

Full attention forward: x->(q,k,v) with l2-normalized weights, per-head-dim
l2 norm + learned qk scale, interleaved RoPE, causal SDPA, output projection
with column-l2-normalized wo.

Sharding: TP=4 over heads (8 heads/core) x DP=2 over batch across 8 cores.
Each core computes a partial [2048, 2048] output for its batch; host sums
the 4 TP partials per batch.

v3 changes vs v2 (from perfetto trace analysis of the 513us baseline):
- PSUM pools split per stream: psL (lg, 2x[128,2,512] = 4 banks),
  psV (pv, 2x[128,512] = 2 banks), psG (proj/yproj/normalize, 2 banks).
  Cross-phase rotation in one shared pool was the main cause of
  group-leader LDWEIGHTS stalls (~100-160ns on ~700 matmuls).
- attn inner loop reordered: exp(p) -> lg4(p+1) -> feed chunks -> pv(p),
  so pv's wait on the Act exp is covered by queued PE work.
- normalize chain (den gather/recip/apply) converted to a generator and
  fed into the NEXT block's attention instead of running exposed after
  the drain; stash/den double-buffered across blocks.
- exp merged to one Act call per (head, sj-pair) also on diagonal pairs
  (the extra 128 garbage cols of the upper plane are never read by pv).
- denominator broadcast matmuls merged per e-tile: 16x [8->128,512]
  instead of 32x [8->64,512].
- Y output in bf16 (host upcasts + sums partials): halves output DMA.
- tail yproj quarters run on psG+psV (4-bank rotation) after attn ends.
- wq/x0 dt=0 tiles split out as small separate DMAs so the first proj
  matmul starts ~5us earlier.
"""
import sys
import os
import math
from contextlib import ExitStack

sys.path.insert(0, "/opt/trn_rl_repo")

import numpy as np
import ml_dtypes

BF16 = ml_dtypes.bfloat16

B, S, DIM = 2, 2048, 2048
HEADS, DH = 32, 64
THETA = 10000.0
NCORES = 8
TP = 4             # head-parallel ways
HPC = HEADS // TP  # heads per core = 8
E = HPC * DH       # per-core qkv width = 512
ET = E // 128      # e-tiles per core = 4
DT = DIM // 128    # contraction d-tiles = 16
SB = S // 512      # 512-wide seq blocks = 4
SS = S // 128      # 128-wide seq blocks = 16

_CACHE = {}


def _l2n(w, axis):
    n = np.sqrt((w.astype(np.float64) ** 2).sum(axis=axis, keepdims=True))
    n = np.maximum(n, 1e-12)
    return (w / n).astype(np.float32)


# split of the 16 contraction d-tiles into DMA tiles: first tile tiny so the
# first matmul's dependencies arrive early.
DT_SPLIT = (1, 3, 4, 4, 4)
DT_START = (0, 1, 4, 8, 12)


def _dt_loc(dt):
    for ti, (s0, n) in enumerate(zip(DT_START, DT_SPLIT)):
        if s0 <= dt < s0 + n:
            return ti, dt - s0
    raise AssertionError


def _build_program():
    import concourse.bass as bass
    from concourse import bacc
    import concourse.mybir as mybir
    import concourse.tile as tile
    from concourse.masks import make_identity

    f32 = mybir.dt.float32
    bf16 = mybir.dt.bfloat16
    AF = mybir.ActivationFunctionType
    AX = mybir.AxisListType
    OP = mybir.AluOpType

    nc = bacc.Bacc("TRN2", target_bir_lowering=False)

    xT = nc.dram_tensor("xT", [DIM, S], bf16, kind="ExternalInput")
    wqT = nc.dram_tensor("wqT", [DIM, E], bf16, kind="ExternalInput")
    wkT = nc.dram_tensor("wkT", [DIM, E], bf16, kind="ExternalInput")
    wvT = nc.dram_tensor("wvT", [DIM, E], bf16, kind="ExternalInput")
    woT = nc.dram_tensor("woT", [E, DIM], bf16, kind="ExternalInput")
    cosd = nc.dram_tensor("cosd", [128, SS * DH], bf16, kind="ExternalInput")
    sind = nc.dram_tensor("sind", [128, SS * DH], bf16, kind="ExternalInput")
    trid = nc.dram_tensor("trid", [128, 128], bf16, kind="ExternalInput")
    ind8d = nc.dram_tensor("ind8d", [8, 512], bf16, kind="ExternalInput")
    Y = nc.dram_tensor("Y", [S, DIM], bf16, kind="ExternalOutput")

    with tile.TileContext(nc) as tc, ExitStack() as ctx:
        const = ctx.enter_context(tc.tile_pool(name="const", bufs=1))
        wpool = ctx.enter_context(tc.tile_pool(name="wpool", bufs=4))
        xpool = ctx.enter_context(tc.tile_pool(name="xpool", bufs=2))
        qkv = ctx.enter_context(tc.tile_pool(name="qkv", bufs=1))
        work = ctx.enter_context(tc.tile_pool(name="work", bufs=1))
        expool = ctx.enter_context(tc.tile_pool(name="expool", bufs=4))
        psL = ctx.enter_context(
            tc.tile_pool(name="psL", bufs=2, space="PSUM"))
        psV = ctx.enter_context(
            tc.tile_pool(name="psV", bufs=2, space="PSUM"))
        psG = ctx.enter_context(
            tc.tile_pool(name="psG", bufs=2, space="PSUM"))

        # --- weights: wq split (tiny dt=0 tile first) so proj starts early ---
        wq_sb = [wpool.tile([128, n, E], bf16, tag=f"wq{j}", bufs=1,
                            name=f"wq{j}")
                 for j, n in enumerate(DT_SPLIT)]
        wk_sb = wpool.tile([128, DT, E], bf16, tag="wk", bufs=1)
        wv_sb = wpool.tile([128, DT, E], bf16, tag="wv", bufs=1)
        wo_sb = wpool.tile([128, ET, DIM], bf16, tag="wo", bufs=1)
        wqr = wqT.rearrange("(t p) e -> p t e", p=128)

        xtiles = {}

        def x0_slice_maker(ts):
            def sl(dt):
                ti, off = _dt_loc(dt)
                return ts[ti][:, off, :]
            return sl

        def x_slice_maker(ts):
            def sl(dt):
                return ts[dt // 4][:, dt % 4, :]
            return sl

        def load_x(st):
            ts = [xpool.tile([128, 4, 512], bf16, tag=f"x{j}", bufs=2,
                             name=f"xst{st}_{j}") for j in range(4)]
            src = xT[:, st * 512:(st + 1) * 512].rearrange(
                "(t p) s -> p t s", p=128)
            for j in range(4):
                nc.sync.dma_start(ts[j], src[:, j * 4:(j + 1) * 4, :])
            return x_slice_maker(ts)

        # st=0 x: split like wq so dt=0 lands first
        x0src = xT[:, 0:512].rearrange("(t p) s -> p t s", p=128)
        x0 = [xpool.tile([128, n, 512], bf16, tag=f"x0s{j}", bufs=1,
                         name=f"xst0_{j}") for j, n in enumerate(DT_SPLIT)]
        # issue the two tiny dt=0 DMAs first
        nc.sync.dma_start(wq_sb[0], wqr[:, 0:1, :])
        nc.sync.dma_start(x0[0], x0src[:, 0:1, :])
        for j in range(1, len(DT_SPLIT)):
            s0, n = DT_START[j], DT_SPLIT[j]
            nc.sync.dma_start(wq_sb[j], wqr[:, s0:s0 + n, :])
            nc.sync.dma_start(x0[j], x0src[:, s0:s0 + n, :])
        xtiles[0] = x0_slice_maker(x0)
        nc.sync.dma_start(wk_sb, wkT.rearrange("(t p) e -> p t e", p=128))
        nc.sync.dma_start(wv_sb, wvT.rearrange("(t p) e -> p t e", p=128))

        def wslice(kind, dt):
            if kind == "q":
                ti, off = _dt_loc(dt)
                return wq_sb[ti][:, off, :]
            return (wk_sb if kind == "k" else wv_sb)[:, dt, :]

        # --- constants ---
        cos_sb = const.tile([128, SS, DH], bf16)
        sin_sb = const.tile([128, SS, DH], bf16)
        nc.sync.dma_start(cos_sb, cosd.rearrange("p (b d) -> p b d", d=DH))
        nc.sync.dma_start(sin_sb, sind.rearrange("p (b d) -> p b d", d=DH))
        tri = const.tile([128, 128], bf16)
        nc.sync.dma_start(tri, trid[:, :])
        ind8 = const.tile([8, 512], bf16)
        nc.sync.dma_start(ind8, ind8d[:, :])
        nc.sync.dma_start(wo_sb, woT.rearrange("(t p) e -> p t e", p=128))
        identf = const.tile([128, 128], f32)
        make_identity(nc, identf)
        ident = const.tile([128, 128], bf16)
        make_identity(nc, ident)

        # --- persistent activations ---
        qTall = qkv.tile([128, ET, S], bf16, tag="qT")
        kTall = qkv.tile([128, ET, S], bf16, tag="kT")
        v_sb = qkv.tile([128, SS, HPC, 66], bf16, tag="v")
        # double-buffered stash: apply(i) overlaps attn(i+1)
        stash = [qkv.tile([65, HPC, 512], bf16, tag=f"stash{m}",
                          name=f"stash{m}")
                 for m in range(2)]
        nc.vector.memset(v_sb[:, :, :, 64:66], 1.0)

        def norm_rope(ps, dstT, st, su):
            """psum [si,e] natural -> per-head l2norm, rope, bf16,
            -> DMA-transpose into dstT columns."""
            sblk = st * 4 + su
            sq = work.tile([128, E], bf16, tag="sq", bufs=2)
            nc.scalar.square(sq, ps)
            ssq = work.tile([128, HPC], f32, tag="ssq", bufs=2)
            nc.vector.tensor_reduce(
                ssq, sq.rearrange("p (h d) -> p h d", d=DH),
                axis=AX.X, op=OP.add)
            # rsqrt via magic-number seed + 2 Newton iterations (DVE only)
            inv = work.tile([128, HPC], f32, tag="inv", bufs=2)
            ssq_i = ssq.bitcast(mybir.dt.int32)
            inv_i = inv.bitcast(mybir.dt.int32)
            nc.vector.tensor_scalar(inv_i, ssq_i, 1, None,
                                    op0=OP.arith_shift_right)
            nc.vector.tensor_scalar(inv_i, inv_i, 0x5f3759df, -1,
                                    op0=OP.subtract, op1=OP.mult)
            y2 = work.tile([128, HPC], f32, tag="y2", bufs=2)
            for _ in range(2):
                nc.vector.tensor_mul(y2, inv, inv)
                nc.vector.scalar_tensor_tensor(
                    y2, ssq, -0.5, y2, op0=OP.mult, op1=OP.mult)
                nc.vector.tensor_scalar(y2, y2, 1.5, None, op0=OP.add)
                nc.vector.tensor_mul(inv, inv, y2)
            qn = work.tile([128, HPC, DH], bf16, tag="qn", bufs=2)
            nc.vector.tensor_mul(
                qn, ps.rearrange("p (h d) -> p h d", d=DH),
                inv.unsqueeze(2).broadcast_to([128, HPC, DH]))
            cosb = cos_sb[:, sblk:sblk + 1, :].broadcast_to([128, HPC, DH])
            sinb = sin_sb[:, sblk:sblk + 1, :].broadcast_to([128, HPC, DH])
            rot = work.tile([128, HPC, 2, 32], bf16, tag="rot", bufs=2)
            qn4 = qn.rearrange("p h (t u) -> p h t u", u=32)
            nc.vector.tensor_copy(rot[:, :, 0:1, :], qn4[:, :, 1:2, :])
            nc.vector.tensor_copy(rot[:, :, 1:2, :], qn4[:, :, 0:1, :])
            nc.vector.tensor_mul(rot.rearrange("p h t u -> p h (t u)"),
                                 rot.rearrange("p h t u -> p h (t u)"), sinb)
            nc.vector.tensor_mul(qn, qn, cosb)
            qo = work.tile([128, E], bf16, tag="qo", bufs=2)
            nc.vector.tensor_add(
                qo, qn.rearrange("p h d -> p (h d)"),
                rot.rearrange("p h t u -> p (h t u)"))
            nc.sync.dma_start_transpose(
                dstT[:, :, sblk * 128:(sblk + 1) * 128], qo)

        def proj_half_gen(kind, st, s0):
            """One su-pair of a proj wave: 4 chunks of 8 matmuls (yields
            between chunks so attention can interleave)."""
            xt = xtiles[st]
            prs = [psG.tile([128, E], f32, tag="g",
                            name=f"p{kind}{st}_{s0 + j}")
                   for j in range(2)]
            for dtc in range(4):
                for dt in range(dtc * 4, dtc * 4 + 4):
                    ws = wslice(kind, dt)
                    for j in range(2):
                        su = s0 + j
                        nc.tensor.matmul(
                            prs[j],
                            xt(dt)[:, su * 128:(su + 1) * 128],
                            ws,
                            start=(dt == 0), stop=(dt == DT - 1))
                if dtc < 3:
                    yield
            for j in range(2):
                su = s0 + j
                if kind == "v":
                    nc.vector.tensor_copy(
                        v_sb[:, st * 4 + su, :, 0:64],
                        prs[j].rearrange("p (h d) -> p h d", d=DH))
                else:
                    norm_rope(prs[j], qTall if kind == "q" else kTall,
                              st, su)

        def proj_half(kind, st, s0):
            for _ in proj_half_gen(kind, st, s0):
                pass

        def proj_all(st):
            for kind in ("q", "k", "v"):
                for s0 in (0, 2):
                    proj_half(kind, st, s0)

        def attn_pair(i, ha, hb, feed):
            """Head-paired attention: heads (h, h+2) share PE tile config.
            Emission order per p: exp(p) -> lg4(p+1) -> feed -> pv(p)."""
            last = 4 * i + 3
            npr = 2 * (i + 1)
            hp = (ha % 2) * 64
            ets = {ha: ha // 2, hb: hb // 2}
            pvs = {h: psV.tile([128, 512], f32, tag="pv",
                               name=f"pv{i}_{h}")
                   for h in (ha, hb)}
            lgs = {}

            def lg4(p):
                for h in (ha, hb):
                    lgs[(h, p)] = psL.tile(
                        [128, 2, 512], f32, tag="lg",
                        name=f"lg{i}_{h}_{p}")
                # h-outer: head a's matmuls aren't queued behind head b's
                # psum-slot wait (slots free per-head as each exp finishes)
                for h in (ha, hb):
                    for b in range(2):
                        sjb = 2 * p + b
                        r = sjb - 4 * i
                        c0 = r * 128 if r > 0 else 0
                        nc.tensor.matmul(
                            lgs[(h, p)][:, b, c0:],
                            kTall[hp:hp + 64, ets[h],
                                  sjb * 128:(sjb + 1) * 128],
                            qTall[hp:hp + 64, ets[h],
                                  i * 512 + c0:(i + 1) * 512],
                            start=True, stop=True)

            lg4(0)
            for p in range(npr):
                diag = 2 * p - 4 * i >= 0
                c0p = max(0, (2 * p - 4 * i)) * 128
                exs = {}
                for h in (ha, hb):
                    lg2 = lgs.pop((h, p))
                    ex = expool.tile([128, 2, 512], bf16, tag="ex",
                                     name=f"ex{i}_{h}_{p}")
                    if diag:
                        # per-plane split: each exp reads only the region
                        # its lg matmul wrote (race-detector clean)
                        for b in range(2):
                            cb = max(0, (2 * p + b - 4 * i)) * 128
                            nc.scalar.activation(ex[:, b, cb:],
                                                 lg2[:, b, cb:], AF.Exp)
                    else:
                        nc.scalar.activation(ex, lg2, AF.Exp)
                    exs[h] = ex
                if p + 1 < npr:
                    lg4(p + 1)
                feed()
                for b in range(2):
                    sjb = 2 * p + b
                    r = sjb - 4 * i
                    c0 = r * 128 if r > 0 else 0
                    if r >= 0:
                        for h in (ha, hb):
                            nc.gpsimd.tensor_mul(
                                exs[h][:, b, r * 128:(r + 1) * 128],
                                exs[h][:, b, r * 128:(r + 1) * 128],
                                tri)
                    for h in (ha, hb):
                        nc.tensor.matmul(
                            pvs[h][0:66, c0:],
                            v_sb[:, sjb, h, :],
                            exs[h][:, b, c0:],
                            start=(sjb == 0), stop=(sjb == last))
            for h in (ha, hb):
                nc.vector.tensor_copy(stash[i % 2][:, h, :], pvs[h][0:65, :])

        def norm_chain_gen(i):
            """den gather -> reciprocal -> per-et broadcast + apply, as a
            feedable generator. Reads stash[i%2], writes qTall cols of i."""
            sb = stash[i % 2]
            den = work.tile([8, 512], bf16, tag="den", bufs=2,
                            name=f"den{i}")
            nc.scalar.dma_start(den, sb[64:65, :, :])
            yield
            invT = psG.tile([128, 32], bf16, tag="g", name=f"invT{i}")
            for c in range(4):
                nc.tensor.transpose(
                    invT[:, c * 8:(c + 1) * 8],
                    den[:, c * 128:(c + 1) * 128], ident[0:8, 0:8])
            inv_sb = work.tile([128, 32], f32, tag="invsb", bufs=2,
                               name=f"invsb{i}")
            nc.vector.reciprocal(inv_sb, invT)
            yield
            invrow = psG.tile([8, 4, 128], f32, tag="g", name=f"invrow{i}")
            for c in range(4):
                nc.tensor.transpose(
                    invrow[:, c, :], inv_sb[:, c * 8:(c + 1) * 8], identf)
            inv_row = work.tile([8, 512], bf16, tag="invrowsb", bufs=2,
                                name=f"invrowsb{i}")
            nc.vector.tensor_copy(
                inv_row, invrow.rearrange("p c j -> p (c j)"))
            yield
            for et in range(ET):
                # [8 -> 128, 512]: partitions 0-63 = head 2et's 1/den,
                # 64-127 = head 2et+1's
                bc = psG.tile([128, 512], f32, tag="g", name=f"bc{i}_{et}")
                nc.tensor.matmul(bc, ind8[:, et * 128:(et + 1) * 128],
                                 inv_row, start=True, stop=True)
                for m in range(2):
                    h = 2 * et + m
                    hp = m * 64
                    nc.vector.tensor_mul(
                        qTall[hp:hp + 64, et, i * 512:(i + 1) * 512],
                        sb[0:64, h, :], bc[hp:hp + 64, :])
                yield

        def yproj_quarter_gen(ib, tail=False):
            pools = (psG, psV) if tail else (psG, psG)
            tags = ("g", "pv") if tail else ("g", "g")
            for nd0 in (0, 2):
                pss = [pools[j].tile([128, 512], f32, tag=tags[j],
                                     name=f"y{ib}_{nd0 + j}")
                       for j in range(2)]
                for ket in range(ET):
                    for j in range(2):
                        nd = nd0 + j
                        nc.tensor.matmul(
                            pss[j],
                            qTall[:, ket, ib * 128:(ib + 1) * 128],
                            wo_sb[:, ket, nd * 512:(nd + 1) * 512],
                            start=(ket == 0), stop=(ket == ET - 1))
                for j in range(2):
                    nd = nd0 + j
                    ys = work.tile([128, 512], bf16, tag="ys", bufs=4,
                                   name=f"ys{ib}_{nd}")
                    if nd % 2 == 0:
                        nc.vector.tensor_copy(ys, pss[j])
                    else:
                        nc.scalar.copy(ys, pss[j])
                    nc.sync.dma_start(
                        Y[ib * 128:(ib + 1) * 128,
                          nd * 512:(nd + 1) * 512], ys)
                if nd0 == 0:
                    yield

        xtiles[1] = load_x(1)
        proj_all(0)
        yq_backlog = []
        pairs = ((0, 2), (4, 6), (1, 3), (5, 7))
        halves = (("q", 0), ("q", 2), ("k", 0), ("k", 2), ("v", 0), ("v", 2))
        for st in range(SB):
            nxt = st + 1 < SB
            if st + 2 < SB:
                xtiles[st + 2] = load_x(st + 2)
            gq = []
            if st > 0:
                gq.append(norm_chain_gen(st - 1))
            if nxt:
                gq += [proj_half_gen(k, st + 1, s) for (k, s) in halves]
            if st > 0:
                yq_backlog.extend(range(4 * (st - 1), 4 * st))
            ntake = (len(yq_backlog) if not nxt
                     else min(2 if st == 1 else 4, len(yq_backlog)))
            for _ in range(ntake):
                gq.append(yproj_quarter_gen(yq_backlog.pop(0)))

            def feed():
                while gq:
                    try:
                        next(gq[0])
                        return
                    except StopIteration:
                        gq.pop(0)

            for ha, hb in pairs:
                attn_pair(st, ha, hb, feed)
            while gq:
                feed()
        # tail: normalize block 3, then its 4 yproj quarters on 4 psum banks
        for g in [norm_chain_gen(SB - 1)] + [
                yproj_quarter_gen(ib, tail=True) for ib in range(12, 16)]:
            for _ in g:
                pass

    return nc


def _host_prep(x, wq, wk, wv, wo, qk_scale):
    """Returns per-core input dicts."""
    perm = np.concatenate([np.arange(0, DH, 2), np.arange(1, DH, 2)])
    wq_n = _l2n(wq, -1).reshape(HEADS, DH, DIM)[:, perm, :].reshape(HEADS * DH, DIM)
    wk_n = _l2n(wk, -1).reshape(HEADS, DH, DIM)[:, perm, :].reshape(HEADS * DH, DIM)
    wv_n = _l2n(wv, -1)
    wo_n = _l2n(wo, 0)
    sp = qk_scale.astype(np.float64)[perm]

    # rope tables with qk_scale folded in; permuted-block layout
    half = np.arange(0, DH, 2)
    freqs = 1.0 / (THETA ** (half.astype(np.float64) / DH))      # (32,)
    ang = np.arange(S, dtype=np.float64)[:, None] * freqs[None]  # (S, 32)
    cos_h, sin_h = np.cos(ang), np.sin(ang)
    cos_p = np.concatenate([cos_h, cos_h], 1)                    # (S, 64)
    sin_e = np.concatenate([-sin_h, sin_h], 1)
    cos_eff = (cos_p * sp[None, :]).astype(np.float32)
    swap_sp = np.concatenate([sp[32:], sp[:32]])
    sin_eff = (sin_e * swap_sp[None, :]).astype(np.float32)
    # device layout [128, SS*DH]: [p, b*64+c] = tbl[b*128+p, c]
    cosd = np.ascontiguousarray(
        cos_eff.reshape(SS, 128, DH).transpose(1, 0, 2).reshape(128, SS * DH))
    sind = np.ascontiguousarray(
        sin_eff.reshape(SS, 128, DH).transpose(1, 0, 2).reshape(128, SS * DH))

    # causal triangle for the diagonal 128-blocks: keep sjl <= sil
    sjl = np.arange(128)[:, None]
    sil = np.arange(128)[None, :]
    trid = (sjl <= sil).astype(np.float32)

    # indicator for denominator broadcast: ind8[k, h*64+m] = (k == h)
    ind8 = np.zeros((8, 512), dtype=np.float32)
    for h in range(8):
        ind8[h, h * 64:(h + 1) * 64] = 1.0

    in_maps = []
    for c in range(NCORES):
        b, t = divmod(c, TP)
        e0 = t * E
        in_maps.append({
            "xT": np.ascontiguousarray(x[b].T).astype(BF16),
            "wqT": np.ascontiguousarray(wq_n[e0:e0 + E].T).astype(BF16),
            "wkT": np.ascontiguousarray(wk_n[e0:e0 + E].T).astype(BF16),
            "wvT": np.ascontiguousarray(wv_n[e0:e0 + E].T).astype(BF16),
            "woT": np.ascontiguousarray(wo_n[:, e0:e0 + E].T).astype(BF16),
            "cosd": cosd.astype(BF16), "sind": sind.astype(BF16),
            "trid": trid.astype(BF16), "ind8d": ind8.astype(BF16),
        })
    return in_maps


def _install_profile_hook():
    """antenv.axon_hooks is absent in this image; shim it and register the
    ctypes NTFF hook against /opt/axon/libaxon_pjrt.so (mirrors trn_boot)."""
    import types
    import ctypes
    import contextlib

    try:
        from antenv.axon_hooks import get_axon_ntff_profile_hook  # noqa
        return
    except ImportError:
        pass
    import antenv
    mod = types.ModuleType("antenv.axon_hooks")
    state = {}
    mod.set_axon_ntff_profile_hook = lambda h: state.__setitem__("h", h)
    mod.get_axon_ntff_profile_hook = lambda: state.get("h")
    sys.modules["antenv.axon_hooks"] = mod
    antenv.axon_hooks = mod

    so_path = "/opt/axon/libaxon_pjrt.so"
    lib = ctypes.CDLL(so_path)
    if not hasattr(lib, "axon_start_nrt_profile"):
        return
    lib.axon_start_nrt_profile.argtypes = [
        ctypes.POINTER(ctypes.c_int64), ctypes.c_size_t]
    lib.axon_start_nrt_profile.restype = ctypes.c_int64
    lib.axon_stop_nrt_profile.argtypes = [ctypes.c_char_p]
    lib.axon_stop_nrt_profile.restype = ctypes.c_int64

    @contextlib.contextmanager
    def _hook(output_dir, device_ids):
        import jax
        jax.devices()
        if device_ids:
            ids = (ctypes.c_int64 * len(device_ids))(*device_ids)
            rc = lib.axon_start_nrt_profile(ids, len(device_ids))
        else:
            rc = lib.axon_start_nrt_profile(None, 0)
        if rc != 0:
            raise RuntimeError(f"axon_start_nrt_profile rc={rc}")
        try:
            yield
        finally:
            n = lib.axon_stop_nrt_profile(str(output_dir).encode())
            print(f"profile: {n} file(s) written to {output_dir}",
                  file=sys.stderr)

    mod.set_axon_ntff_profile_hook(_hook)


def kernel(x, wq, wk, wv, wo, qk_scale, _profile=False):
    from concourse.bass_utils import run_bass_kernel_spmd

    if _profile:
        _install_profile_hook()

    if "nc" not in _CACHE:
        nc = _build_program()
        nc.finalize()
        _CACHE["nc"] = nc
    nc = _CACHE["nc"]
    in_maps = _host_prep(np.asarray(x), np.asarray(wq), np.asarray(wk),
                         np.asarray(wv), np.asarray(wo), np.asarray(qk_scale))
    res = run_bass_kernel_spmd(nc, in_maps, core_ids=list(range(NCORES)),
                               trace=_profile)
    outs = res.results
    y = np.empty((B, S, DIM), dtype=np.float32)
    for b in range(B):
        y[b] = sum(np.asarray(outs[b * TP + t]["Y"], dtype=np.float32)
                   for t in range(TP))
    if _profile:
        _CACHE["last_exec_time_ns"] = res.exec_time_ns
        _CACHE["last_profile"] = res.profile_json
    return y


# revision 13
# speedup vs baseline: 1.0552x; 1.0035x over previous
"""nn_Attention Trainium2 Bass kernel (v3 — stall-free scheduling).

Full attention forward: x->(q,k,v) with l2-normalized weights, per-head-dim
l2 norm + learned qk scale, interleaved RoPE, causal SDPA, output projection
with column-l2-normalized wo.

Sharding: TP=4 over heads (8 heads/core) x DP=2 over batch across 8 cores.
Each core computes a partial [2048, 2048] output for its batch; host sums
the 4 TP partials per batch.

v3 changes vs v2 (from perfetto trace analysis of the 513us baseline):
- PSUM pools split per stream: psL (lg, 2x[128,2,512] = 4 banks),
  psV (pv, 2x[128,512] = 2 banks), psG (proj/yproj/normalize, 2 banks).
  Cross-phase rotation in one shared pool was the main cause of
  group-leader LDWEIGHTS stalls (~100-160ns on ~700 matmuls).
- attn inner loop reordered: exp(p) -> lg4(p+1) -> feed chunks -> pv(p),
  so pv's wait on the Act exp is covered by queued PE work.
- normalize chain (den gather/recip/apply) converted to a generator and
  fed into the NEXT block's attention instead of running exposed after
  the drain; stash/den double-buffered across blocks.
- exp merged to one Act call per (head, sj-pair) also on diagonal pairs
  (the extra 128 garbage cols of the upper plane are never read by pv).
- denominator broadcast matmuls merged per e-tile: 16x [8->128,512]
  instead of 32x [8->64,512].
- Y output in bf16 (host upcasts + sums partials): halves output DMA.
- tail yproj quarters run on psG+psV (4-bank rotation) after attn ends.
- wq/x0 dt=0 tiles split out as small separate DMAs so the first proj
  matmul starts ~5us earlier.
"""
import sys
import os
import math
from contextlib import ExitStack

sys.path.insert(0, "/opt/trn_rl_repo")

import numpy as np
import ml_dtypes

BF16 = ml_dtypes.bfloat16

B, S, DIM = 2, 2048, 2048
HEADS, DH = 32, 64
THETA = 10000.0
NCORES = 8
TP = 4             # head-parallel ways
HPC = HEADS // TP  # heads per core = 8
E = HPC * DH       # per-core qkv width = 512
ET = E // 128      # e-tiles per core = 4
DT = DIM // 128    # contraction d-tiles = 16
SB = S // 512      # 512-wide seq blocks = 4
SS = S // 128      # 128-wide seq blocks = 16

_CACHE = {}


def _l2n(w, axis):
    n = np.sqrt((w.astype(np.float64) ** 2).sum(axis=axis, keepdims=True))
    n = np.maximum(n, 1e-12)
    return (w / n).astype(np.float32)


# split of the 16 contraction d-tiles into DMA tiles: first tile tiny so the
# first matmul's dependencies arrive early.
DT_SPLIT = (1, 3, 4, 4, 4)
DT_START = (0, 1, 4, 8, 12)


def _dt_loc(dt):
    for ti, (s0, n) in enumerate(zip(DT_START, DT_SPLIT)):
        if s0 <= dt < s0 + n:
            return ti, dt - s0
    raise AssertionError


def _build_program():
    import concourse.bass as bass
    from concourse import bacc
    import concourse.mybir as mybir
    import concourse.tile as tile
    from concourse.masks import make_identity

    f32 = mybir.dt.float32
    bf16 = mybir.dt.bfloat16
    AF = mybir.ActivationFunctionType
    AX = mybir.AxisListType
    OP = mybir.AluOpType

    nc = bacc.Bacc("TRN2", target_bir_lowering=False)

    xT = nc.dram_tensor("xT", [DIM, S], bf16, kind="ExternalInput")
    wqT = nc.dram_tensor("wqT", [DIM, E], bf16, kind="ExternalInput")
    wkT = nc.dram_tensor("wkT", [DIM, E], bf16, kind="ExternalInput")
    wvT = nc.dram_tensor("wvT", [DIM, E], bf16, kind="ExternalInput")
    woT = nc.dram_tensor("woT", [E, DIM], bf16, kind="ExternalInput")
    cosd = nc.dram_tensor("cosd", [128, SS * DH], bf16, kind="ExternalInput")
    sind = nc.dram_tensor("sind", [128, SS * DH], bf16, kind="ExternalInput")
    trid = nc.dram_tensor("trid", [128, 128], bf16, kind="ExternalInput")
    ind8d = nc.dram_tensor("ind8d", [8, 512], bf16, kind="ExternalInput")
    Y = nc.dram_tensor("Y", [S, DIM], bf16, kind="ExternalOutput")

    with tile.TileContext(nc) as tc, ExitStack() as ctx:
        const = ctx.enter_context(tc.tile_pool(name="const", bufs=1))
        wpool = ctx.enter_context(tc.tile_pool(name="wpool", bufs=4))
        xpool = ctx.enter_context(tc.tile_pool(name="xpool", bufs=2))
        qkv = ctx.enter_context(tc.tile_pool(name="qkv", bufs=1))
        work = ctx.enter_context(tc.tile_pool(name="work", bufs=1))
        expool = ctx.enter_context(tc.tile_pool(name="expool", bufs=4))
        psL = ctx.enter_context(
            tc.tile_pool(name="psL", bufs=2, space="PSUM"))
        psV = ctx.enter_context(
            tc.tile_pool(name="psV", bufs=2, space="PSUM"))
        psG = ctx.enter_context(
            tc.tile_pool(name="psG", bufs=2, space="PSUM"))

        # --- weights: wq split (tiny dt=0 tile first) so proj starts early ---
        wq_sb = [wpool.tile([128, n, E], bf16, tag=f"wq{j}", bufs=1,
                            name=f"wq{j}")
                 for j, n in enumerate(DT_SPLIT)]
        wk_sb = wpool.tile([128, DT, E], bf16, tag="wk", bufs=1)
        wv_sb = wpool.tile([128, DT, E], bf16, tag="wv", bufs=1)
        wo_sb = wpool.tile([128, ET, DIM], bf16, tag="wo", bufs=1)
        wqr = wqT.rearrange("(t p) e -> p t e", p=128)

        xtiles = {}

        def x0_slice_maker(ts):
            def sl(dt):
                ti, off = _dt_loc(dt)
                return ts[ti][:, off, :]
            return sl

        def x_slice_maker(ts):
            def sl(dt):
                return ts[dt // 4][:, dt % 4, :]
            return sl

        def load_x(st):
            ts = [xpool.tile([128, 4, 512], bf16, tag=f"x{j}", bufs=2,
                             name=f"xst{st}_{j}") for j in range(4)]
            src = xT[:, st * 512:(st + 1) * 512].rearrange(
                "(t p) s -> p t s", p=128)
            for j in range(4):
                nc.sync.dma_start(ts[j], src[:, j * 4:(j + 1) * 4, :])
            return x_slice_maker(ts)

        # st=0 x: split like wq so dt=0 lands first
        x0src = xT[:, 0:512].rearrange("(t p) s -> p t s", p=128)
        x0 = [xpool.tile([128, n, 512], bf16, tag=f"x0s{j}", bufs=1,
                         name=f"xst0_{j}") for j, n in enumerate(DT_SPLIT)]
        # issue the two tiny dt=0 DMAs first
        nc.sync.dma_start(wq_sb[0], wqr[:, 0:1, :])
        nc.sync.dma_start(x0[0], x0src[:, 0:1, :])
        for j in range(1, len(DT_SPLIT)):
            s0, n = DT_START[j], DT_SPLIT[j]
            nc.sync.dma_start(wq_sb[j], wqr[:, s0:s0 + n, :])
            nc.sync.dma_start(x0[j], x0src[:, s0:s0 + n, :])
        xtiles[0] = x0_slice_maker(x0)
        nc.sync.dma_start(wk_sb, wkT.rearrange("(t p) e -> p t e", p=128))
        nc.sync.dma_start(wv_sb, wvT.rearrange("(t p) e -> p t e", p=128))

        def wslice(kind, dt):
            if kind == "q":
                ti, off = _dt_loc(dt)
                return wq_sb[ti][:, off, :]
            return (wk_sb if kind == "k" else wv_sb)[:, dt, :]

        # --- constants ---
        cos_sb = const.tile([128, SS, DH], bf16)
        sin_sb = const.tile([128, SS, DH], bf16)
        nc.sync.dma_start(cos_sb, cosd.rearrange("p (b d) -> p b d", d=DH))
        nc.sync.dma_start(sin_sb, sind.rearrange("p (b d) -> p b d", d=DH))
        tri = const.tile([128, 128], bf16)
        nc.sync.dma_start(tri, trid[:, :])
        ind8 = const.tile([8, 512], bf16)
        nc.sync.dma_start(ind8, ind8d[:, :])
        nc.sync.dma_start(wo_sb, woT.rearrange("(t p) e -> p t e", p=128))
        identf = const.tile([128, 128], f32)
        make_identity(nc, identf)
        ident = const.tile([128, 128], bf16)
        make_identity(nc, ident)

        # --- persistent activations ---
        qTall = qkv.tile([128, ET, S], bf16, tag="qT")
        kTall = qkv.tile([128, ET, S], bf16, tag="kT")
        v_sb = qkv.tile([128, SS, HPC, 66], bf16, tag="v")
        # double-buffered stash: apply(i) overlaps attn(i+1)
        stash = [qkv.tile([65, HPC, 512], bf16, tag=f"stash{m}",
                          name=f"stash{m}")
                 for m in range(2)]
        nc.vector.memset(v_sb[:, :, :, 64:66], 1.0)

        def norm_rope(ps, dstT, st, su):
            """psum [si,e] natural -> per-head l2norm, rope, bf16,
            -> DMA-transpose into dstT columns."""
            sblk = st * 4 + su
            sq = work.tile([128, E], bf16, tag="sq", bufs=2)
            nc.scalar.square(sq, ps)
            ssq = work.tile([128, HPC], f32, tag="ssq", bufs=2)
            nc.vector.tensor_reduce(
                ssq, sq.rearrange("p (h d) -> p h d", d=DH),
                axis=AX.X, op=OP.add)
            # rsqrt via magic-number seed + 2 Newton iterations (DVE only)
            inv = work.tile([128, HPC], f32, tag="inv", bufs=2)
            ssq_i = ssq.bitcast(mybir.dt.int32)
            inv_i = inv.bitcast(mybir.dt.int32)
            nc.vector.tensor_scalar(inv_i, ssq_i, 1, None,
                                    op0=OP.arith_shift_right)
            nc.vector.tensor_scalar(inv_i, inv_i, 0x5f3759df, -1,
                                    op0=OP.subtract, op1=OP.mult)
            y2 = work.tile([128, HPC], f32, tag="y2", bufs=2)
            for _ in range(2):
                nc.vector.tensor_mul(y2, inv, inv)
                nc.vector.scalar_tensor_tensor(
                    y2, ssq, -0.5, y2, op0=OP.mult, op1=OP.mult)
                nc.vector.tensor_scalar(y2, y2, 1.5, None, op0=OP.add)
                nc.vector.tensor_mul(inv, inv, y2)
            qn = work.tile([128, HPC, DH], bf16, tag="qn", bufs=2)
            nc.vector.tensor_mul(
                qn, ps.rearrange("p (h d) -> p h d", d=DH),
                inv.unsqueeze(2).broadcast_to([128, HPC, DH]))
            cosb = cos_sb[:, sblk:sblk + 1, :].broadcast_to([128, HPC, DH])
            sinb = sin_sb[:, sblk:sblk + 1, :].broadcast_to([128, HPC, DH])
            rot = work.tile([128, HPC, 2, 32], bf16, tag="rot", bufs=2)
            qn4 = qn.rearrange("p h (t u) -> p h t u", u=32)
            nc.vector.tensor_copy(rot[:, :, 0:1, :], qn4[:, :, 1:2, :])
            nc.vector.tensor_copy(rot[:, :, 1:2, :], qn4[:, :, 0:1, :])
            nc.vector.tensor_mul(rot.rearrange("p h t u -> p h (t u)"),
                                 rot.rearrange("p h t u -> p h (t u)"), sinb)
            nc.vector.tensor_mul(qn, qn, cosb)
            qo = work.tile([128, E], bf16, tag="qo", bufs=2)
            nc.vector.tensor_add(
                qo, qn.rearrange("p h d -> p (h d)"),
                rot.rearrange("p h t u -> p (h t u)"))
            nc.sync.dma_start_transpose(
                dstT[:, :, sblk * 128:(sblk + 1) * 128], qo)

        def proj_half_gen(kind, st, s0, wide=False):
            """One su-pair of a proj wave: 4 chunks of 8 matmuls (yields
            between chunks so attention can interleave). wide=True uses
            psG+psV (4-bank rotation) for the pre-attention wave."""
            xt = xtiles[st]
            pools = (psG, psV) if wide else (psG, psG)
            tags = ("g", "pv") if wide else ("g", "g")
            prs = [pools[j].tile([128, E], f32, tag=tags[j],
                                 name=f"p{kind}{st}_{s0 + j}")
                   for j in range(2)]
            for dtc in range(4):
                for dt in range(dtc * 4, dtc * 4 + 4):
                    ws = wslice(kind, dt)
                    for j in range(2):
                        su = s0 + j
                        nc.tensor.matmul(
                            prs[j],
                            xt(dt)[:, su * 128:(su + 1) * 128],
                            ws,
                            start=(dt == 0), stop=(dt == DT - 1))
                if dtc < 3:
                    yield
            for j in range(2):
                su = s0 + j
                if kind == "v":
                    nc.vector.tensor_copy(
                        v_sb[:, st * 4 + su, :, 0:64],
                        prs[j].rearrange("p (h d) -> p h d", d=DH))
                else:
                    norm_rope(prs[j], qTall if kind == "q" else kTall,
                              st, su)

        def proj_half(kind, st, s0, wide=False):
            for _ in proj_half_gen(kind, st, s0, wide):
                pass

        def proj_all(st):
            for kind in ("q", "k", "v"):
                for s0 in (0, 2):
                    proj_half(kind, st, s0, wide=True)

        def attn_pair(i, ha, hb, feed):
            """Head-paired attention: heads (h, h+2) share PE tile config.
            Emission order per p: exp(p) -> lg4(p+1) -> feed -> pv(p)."""
            last = 4 * i + 3
            npr = 2 * (i + 1)
            hp = (ha % 2) * 64
            ets = {ha: ha // 2, hb: hb // 2}
            pvs = {h: psV.tile([128, 512], f32, tag="pv",
                               name=f"pv{i}_{h}")
                   for h in (ha, hb)}
            lgs = {}

            def lg4(p):
                for h in (ha, hb):
                    lgs[(h, p)] = psL.tile(
                        [128, 2, 512], f32, tag="lg",
                        name=f"lg{i}_{h}_{p}")
                # h-outer: head a's matmuls aren't queued behind head b's
                # psum-slot wait (slots free per-head as each exp finishes)
                for h in (ha, hb):
                    for b in range(2):
                        sjb = 2 * p + b
                        r = sjb - 4 * i
                        c0 = r * 128 if r > 0 else 0
                        nc.tensor.matmul(
                            lgs[(h, p)][:, b, c0:],
                            kTall[hp:hp + 64, ets[h],
                                  sjb * 128:(sjb + 1) * 128],
                            qTall[hp:hp + 64, ets[h],
                                  i * 512 + c0:(i + 1) * 512],
                            start=True, stop=True)

            lg4(0)
            for p in range(npr):
                diag = 2 * p - 4 * i >= 0
                c0p = max(0, (2 * p - 4 * i)) * 128
                exs = {}
                for h in (ha, hb):
                    lg2 = lgs.pop((h, p))
                    ex = expool.tile([128, 2, 512], bf16, tag="ex",
                                     name=f"ex{i}_{h}_{p}")
                    if diag:
                        # per-plane split: each exp reads only the region
                        # its lg matmul wrote (race-detector clean)
                        for b in range(2):
                            cb = max(0, (2 * p + b - 4 * i)) * 128
                            nc.scalar.activation(ex[:, b, cb:],
                                                 lg2[:, b, cb:], AF.Exp)
                    else:
                        nc.scalar.activation(ex, lg2, AF.Exp)
                    exs[h] = ex
                if p + 1 < npr:
                    lg4(p + 1)
                feed()
                if i == 0:
                    feed()
                for b in range(2):
                    sjb = 2 * p + b
                    r = sjb - 4 * i
                    c0 = r * 128 if r > 0 else 0
                    if r >= 0:
                        for h in (ha, hb):
                            nc.gpsimd.tensor_mul(
                                exs[h][:, b, r * 128:(r + 1) * 128],
                                exs[h][:, b, r * 128:(r + 1) * 128],
                                tri)
                    for h in (ha, hb):
                        nc.tensor.matmul(
                            pvs[h][0:66, c0:],
                            v_sb[:, sjb, h, :],
                            exs[h][:, b, c0:],
                            start=(sjb == 0), stop=(sjb == last))
            # split across engines: a late stash copy delays the next
            # pair's psV slot (GpSimd can't read PSUM, so DVE + Act; Act
            # gets an even partition count — 65 came back corrupted)
            nc.vector.tensor_copy(stash[i % 2][:, ha, :], pvs[ha][0:65, :])
            nc.scalar.copy(stash[i % 2][0:64, hb, :], pvs[hb][0:64, :])
            nc.vector.tensor_copy(stash[i % 2][64:65, hb, :],
                                  pvs[hb][64:65, :])

        def norm_chain_gen(i):
            """den gather -> reciprocal -> per-et broadcast + apply, as a
            feedable generator. Reads stash[i%2], writes qTall cols of i."""
            sb = stash[i % 2]
            den = work.tile([8, 512], bf16, tag="den", bufs=2,
                            name=f"den{i}")
            nc.scalar.dma_start(den, sb[64:65, :, :])
            yield
            invT = psG.tile([128, 32], bf16, tag="g", name=f"invT{i}")
            for c in range(4):
                nc.tensor.transpose(
                    invT[:, c * 8:(c + 1) * 8],
                    den[:, c * 128:(c + 1) * 128], ident[0:8, 0:8])
            inv_sb = work.tile([128, 32], f32, tag="invsb", bufs=2,
                               name=f"invsb{i}")
            nc.vector.reciprocal(inv_sb, invT)
            yield
            invrow = psG.tile([8, 4, 128], f32, tag="g", name=f"invrow{i}")
            for c in range(4):
                nc.tensor.transpose(
                    invrow[:, c, :], inv_sb[:, c * 8:(c + 1) * 8], identf)
            inv_row = work.tile([8, 512], bf16, tag="invrowsb", bufs=2,
                                name=f"invrowsb{i}")
            nc.vector.tensor_copy(
                inv_row, invrow.rearrange("p c j -> p (c j)"))
            yield
            for et in range(ET):
                # [8 -> 128, 512]: partitions 0-63 = head 2et's 1/den,
                # 64-127 = head 2et+1's
                bc = psG.tile([128, 512], f32, tag="g", name=f"bc{i}_{et}")
                nc.tensor.matmul(bc, ind8[:, et * 128:(et + 1) * 128],
                                 inv_row, start=True, stop=True)
                for m in range(2):
                    h = 2 * et + m
                    hp = m * 64
                    nc.vector.tensor_mul(
                        qTall[hp:hp + 64, et, i * 512:(i + 1) * 512],
                        sb[0:64, h, :], bc[hp:hp + 64, :])
                yield

        def yproj_quarter_gen(ib, tail=False):
            pools = (psG, psV) if tail else (psG, psG)
            tags = ("g", "pv") if tail else ("g", "g")
            for nd0 in (0, 2):
                pss = [pools[j].tile([128, 512], f32, tag=tags[j],
                                     name=f"y{ib}_{nd0 + j}")
                       for j in range(2)]
                for ket in range(ET):
                    for j in range(2):
                        nd = nd0 + j
                        nc.tensor.matmul(
                            pss[j],
                            qTall[:, ket, ib * 128:(ib + 1) * 128],
                            wo_sb[:, ket, nd * 512:(nd + 1) * 512],
                            start=(ket == 0), stop=(ket == ET - 1))
                for j in range(2):
                    nd = nd0 + j
                    ys = work.tile([128, 512], bf16, tag="ys", bufs=4,
                                   name=f"ys{ib}_{nd}")
                    if nd % 2 == 0:
                        nc.vector.tensor_copy(ys, pss[j])
                    else:
                        nc.scalar.copy(ys, pss[j])
                    nc.sync.dma_start(
                        Y[ib * 128:(ib + 1) * 128,
                          nd * 512:(nd + 1) * 512], ys)
                if nd0 == 0:
                    yield

        xtiles[1] = load_x(1)
        proj_all(0)
        yq_backlog = []
        pairs = ((0, 2), (4, 6), (1, 3), (5, 7))
        halves = (("q", 0), ("q", 2), ("k", 0), ("k", 2), ("v", 0), ("v", 2))
        for st in range(SB):
            nxt = st + 1 < SB
            if st + 2 < SB:
                xtiles[st + 2] = load_x(st + 2)
            gq = []
            if st > 0:
                gq.append(norm_chain_gen(st - 1))
            if nxt:
                gq += [proj_half_gen(k, st + 1, s) for (k, s) in halves]
            if st > 0:
                yq_backlog.extend(range(4 * (st - 1), 4 * st))
            ntake = (len(yq_backlog) if not nxt
                     else min(2 if st == 1 else 4, len(yq_backlog)))
            for _ in range(ntake):
                gq.append(yproj_quarter_gen(yq_backlog.pop(0)))

            def feed():
                while gq:
                    try:
                        next(gq[0])
                        return
                    except StopIteration:
                        gq.pop(0)

            for ha, hb in pairs:
                attn_pair(st, ha, hb, feed)
            while gq:
                feed()
        # tail: normalize block 3, then its 4 yproj quarters on 4 psum banks
        for g in [norm_chain_gen(SB - 1)] + [
                yproj_quarter_gen(ib, tail=True) for ib in range(12, 16)]:
            for _ in g:
                pass

    return nc


def _host_prep(x, wq, wk, wv, wo, qk_scale):
    """Returns per-core input dicts."""
    perm = np.concatenate([np.arange(0, DH, 2), np.arange(1, DH, 2)])
    wq_n = _l2n(wq, -1).reshape(HEADS, DH, DIM)[:, perm, :].reshape(HEADS * DH, DIM)
    wk_n = _l2n(wk, -1).reshape(HEADS, DH, DIM)[:, perm, :].reshape(HEADS * DH, DIM)
    wv_n = _l2n(wv, -1)
    wo_n = _l2n(wo, 0)
    sp = qk_scale.astype(np.float64)[perm]

    # rope tables with qk_scale folded in; permuted-block layout
    half = np.arange(0, DH, 2)
    freqs = 1.0 / (THETA ** (half.astype(np.float64) / DH))      # (32,)
    ang = np.arange(S, dtype=np.float64)[:, None] * freqs[None]  # (S, 32)
    cos_h, sin_h = np.cos(ang), np.sin(ang)
    cos_p = np.concatenate([cos_h, cos_h], 1)                    # (S, 64)
    sin_e = np.concatenate([-sin_h, sin_h], 1)
    cos_eff = (cos_p * sp[None, :]).astype(np.float32)
    swap_sp = np.concatenate([sp[32:], sp[:32]])
    sin_eff = (sin_e * swap_sp[None, :]).astype(np.float32)
    # device layout [128, SS*DH]: [p, b*64+c] = tbl[b*128+p, c]
    cosd = np.ascontiguousarray(
        cos_eff.reshape(SS, 128, DH).transpose(1, 0, 2).reshape(128, SS * DH))
    sind = np.ascontiguousarray(
        sin_eff.reshape(SS, 128, DH).transpose(1, 0, 2).reshape(128, SS * DH))

    # causal triangle for the diagonal 128-blocks: keep sjl <= sil
    sjl = np.arange(128)[:, None]
    sil = np.arange(128)[None, :]
    trid = (sjl <= sil).astype(np.float32)

    # indicator for denominator broadcast: ind8[k, h*64+m] = (k == h)
    ind8 = np.zeros((8, 512), dtype=np.float32)
    for h in range(8):
        ind8[h, h * 64:(h + 1) * 64] = 1.0

    in_maps = []
    for c in range(NCORES):
        b, t = divmod(c, TP)
        e0 = t * E
        in_maps.append({
            "xT": np.ascontiguousarray(x[b].T).astype(BF16),
            "wqT": np.ascontiguousarray(wq_n[e0:e0 + E].T).astype(BF16),
            "wkT": np.ascontiguousarray(wk_n[e0:e0 + E].T).astype(BF16),
            "wvT": np.ascontiguousarray(wv_n[e0:e0 + E].T).astype(BF16),
            "woT": np.ascontiguousarray(wo_n[:, e0:e0 + E].T).astype(BF16),
            "cosd": cosd.astype(BF16), "sind": sind.astype(BF16),
            "trid": trid.astype(BF16), "ind8d": ind8.astype(BF16),
        })
    return in_maps


def _install_profile_hook():
    """antenv.axon_hooks is absent in this image; shim it and register the
    ctypes NTFF hook against /opt/axon/libaxon_pjrt.so (mirrors trn_boot)."""
    import types
    import ctypes
    import contextlib

    try:
        from antenv.axon_hooks import get_axon_ntff_profile_hook  # noqa
        return
    except ImportError:
        pass
    import antenv
    mod = types.ModuleType("antenv.axon_hooks")
    state = {}
    mod.set_axon_ntff_profile_hook = lambda h: state.__setitem__("h", h)
    mod.get_axon_ntff_profile_hook = lambda: state.get("h")
    sys.modules["antenv.axon_hooks"] = mod
    antenv.axon_hooks = mod

    so_path = "/opt/axon/libaxon_pjrt.so"
    lib = ctypes.CDLL(so_path)
    if not hasattr(lib, "axon_start_nrt_profile"):
        return
    lib.axon_start_nrt_profile.argtypes = [
        ctypes.POINTER(ctypes.c_int64), ctypes.c_size_t]
    lib.axon_start_nrt_profile.restype = ctypes.c_int64
    lib.axon_stop_nrt_profile.argtypes = [ctypes.c_char_p]
    lib.axon_stop_nrt_profile.restype = ctypes.c_int64

    @contextlib.contextmanager
    def _hook(output_dir, device_ids):
        import jax
        jax.devices()
        if device_ids:
            ids = (ctypes.c_int64 * len(device_ids))(*device_ids)
            rc = lib.axon_start_nrt_profile(ids, len(device_ids))
        else:
            rc = lib.axon_start_nrt_profile(None, 0)
        if rc != 0:
            raise RuntimeError(f"axon_start_nrt_profile rc={rc}")
        try:
            yield
        finally:
            n = lib.axon_stop_nrt_profile(str(output_dir).encode())
            print(f"profile: {n} file(s) written to {output_dir}",
                  file=sys.stderr)

    mod.set_axon_ntff_profile_hook(_hook)


def kernel(x, wq, wk, wv, wo, qk_scale, _profile=False):
    from concourse.bass_utils import run_bass_kernel_spmd

    if _profile:
        _install_profile_hook()

    if "nc" not in _CACHE:
        nc = _build_program()
        nc.finalize()
        _CACHE["nc"] = nc
    nc = _CACHE["nc"]
    in_maps = _host_prep(np.asarray(x), np.asarray(wq), np.asarray(wk),
                         np.asarray(wv), np.asarray(wo), np.asarray(qk_scale))
    res = run_bass_kernel_spmd(nc, in_maps, core_ids=list(range(NCORES)),
                               trace=_profile)
    outs = res.results
    y = np.empty((B, S, DIM), dtype=np.float32)
    for b in range(B):
        y[b] = sum(np.asarray(outs[b * TP + t]["Y"], dtype=np.float32)
                   for t in range(TP))
    if _profile:
        _CACHE["last_exec_time_ns"] = res.exec_time_ns
        _CACHE["last_profile"] = res.profile_json
    return y


# revision 19
# speedup vs baseline: 1.0946x; 1.0373x over previous
"""nn_Attention Trainium2 Bass kernel (v3 — stall-free scheduling).

Full attention forward: x->(q,k,v) with l2-normalized weights, per-head-dim
l2 norm + learned qk scale, interleaved RoPE, causal SDPA, output projection
with column-l2-normalized wo.

Sharding: TP=4 over heads (8 heads/core) x DP=2 over batch across 8 cores.
Each core computes a partial [2048, 2048] output for its batch; host sums
the 4 TP partials per batch.

v3 changes vs v2 (from perfetto trace analysis of the 513us baseline):
- PSUM pools split per stream: psL (lg, 2x[128,2,512] = 4 banks),
  psV (pv, 2x[128,512] = 2 banks), psG (proj/yproj/normalize, 2 banks).
  Cross-phase rotation in one shared pool was the main cause of
  group-leader LDWEIGHTS stalls (~100-160ns on ~700 matmuls).
- attn inner loop reordered: exp(p) -> lg4(p+1) -> feed chunks -> pv(p),
  so pv's wait on the Act exp is covered by queued PE work.
- normalize chain (den gather/recip/apply) converted to a generator and
  fed into the NEXT block's attention instead of running exposed after
  the drain; stash/den double-buffered across blocks.
- exp merged to one Act call per (head, sj-pair) also on diagonal pairs
  (the extra 128 garbage cols of the upper plane are never read by pv).
- denominator broadcast matmuls merged per e-tile: 16x [8->128,512]
  instead of 32x [8->64,512].
- Y output in bf16 (host upcasts + sums partials): halves output DMA.
- tail yproj quarters run on psG+psV (4-bank rotation) after attn ends.
- wq/x0 dt=0 tiles split out as small separate DMAs so the first proj
  matmul starts ~5us earlier.
"""
import sys
import os
import math
from contextlib import ExitStack

sys.path.insert(0, "/opt/trn_rl_repo")

import numpy as np
import ml_dtypes

BF16 = ml_dtypes.bfloat16

B, S, DIM = 2, 2048, 2048
HEADS, DH = 32, 64
THETA = 10000.0
NCORES = 8
TP = 4             # head-parallel ways
HPC = HEADS // TP  # heads per core = 8
E = HPC * DH       # per-core qkv width = 512
ET = E // 128      # e-tiles per core = 4
DT = DIM // 128    # contraction d-tiles = 16
SB = S // 512      # 512-wide seq blocks = 4
SS = S // 128      # 128-wide seq blocks = 16

_CACHE = {}


def _l2n(w, axis):
    n = np.sqrt((w.astype(np.float64) ** 2).sum(axis=axis, keepdims=True))
    n = np.maximum(n, 1e-12)
    return (w / n).astype(np.float32)


# split of the 16 contraction d-tiles into DMA tiles: first tile tiny so the
# first matmul's dependencies arrive early.
DT_SPLIT = (1, 3, 4, 4, 4)
DT_START = (0, 1, 4, 8, 12)


def _dt_loc(dt):
    for ti, (s0, n) in enumerate(zip(DT_START, DT_SPLIT)):
        if s0 <= dt < s0 + n:
            return ti, dt - s0
    raise AssertionError


def _build_program():
    import concourse.bass as bass
    from concourse import bacc
    import concourse.mybir as mybir
    import concourse.tile as tile
    from concourse.masks import make_identity

    f32 = mybir.dt.float32
    bf16 = mybir.dt.bfloat16
    AF = mybir.ActivationFunctionType
    AX = mybir.AxisListType
    OP = mybir.AluOpType

    nc = bacc.Bacc("TRN2", target_bir_lowering=False)

    xT = nc.dram_tensor("xT", [DIM, S], bf16, kind="ExternalInput")
    wqT = nc.dram_tensor("wqT", [DIM, E], bf16, kind="ExternalInput")
    wkT = nc.dram_tensor("wkT", [DIM, E], bf16, kind="ExternalInput")
    wvT = nc.dram_tensor("wvT", [DIM, E], bf16, kind="ExternalInput")
    woT = nc.dram_tensor("woT", [E, DIM], bf16, kind="ExternalInput")
    cosd = nc.dram_tensor("cosd", [128, SS * DH], bf16, kind="ExternalInput")
    sind = nc.dram_tensor("sind", [128, SS * DH], bf16, kind="ExternalInput")
    trid = nc.dram_tensor("trid", [128, 128], bf16, kind="ExternalInput")
    ind8d = nc.dram_tensor("ind8d", [8, 512], bf16, kind="ExternalInput")
    Y = nc.dram_tensor("Y", [S, DIM], bf16, kind="ExternalOutput")

    with tile.TileContext(nc) as tc, ExitStack() as ctx:
        const = ctx.enter_context(tc.tile_pool(name="const", bufs=1))
        wpool = ctx.enter_context(tc.tile_pool(name="wpool", bufs=4))
        xpool = ctx.enter_context(tc.tile_pool(name="xpool", bufs=2))
        qkv = ctx.enter_context(tc.tile_pool(name="qkv", bufs=1))
        work = ctx.enter_context(tc.tile_pool(name="work", bufs=1))
        expool = ctx.enter_context(tc.tile_pool(name="expool", bufs=4))
        psL = ctx.enter_context(
            tc.tile_pool(name="psL", bufs=2, space="PSUM"))
        psV = ctx.enter_context(
            tc.tile_pool(name="psV", bufs=2, space="PSUM"))
        psG = ctx.enter_context(
            tc.tile_pool(name="psG", bufs=2, space="PSUM"))

        # --- weights: wq split (tiny dt=0 tile first) so proj starts early ---
        wq_sb = [wpool.tile([128, n, E], bf16, tag=f"wq{j}", bufs=1,
                            name=f"wq{j}")
                 for j, n in enumerate(DT_SPLIT)]
        wk_sb = wpool.tile([128, DT, E], bf16, tag="wk", bufs=1)
        wv_sb = wpool.tile([128, DT, E], bf16, tag="wv", bufs=1)
        wo_sb = wpool.tile([128, ET, DIM], bf16, tag="wo", bufs=1)
        wqr = wqT.rearrange("(t p) e -> p t e", p=128)

        xtiles = {}

        def x0_slice_maker(ts):
            def sl(dt):
                ti, off = _dt_loc(dt)
                return ts[ti][:, off, :]
            return sl

        def x_slice_maker(ts):
            def sl(dt):
                return ts[dt // 4][:, dt % 4, :]
            return sl

        def load_x(st):
            ts = [xpool.tile([128, 4, 512], bf16, tag=f"x{j}", bufs=2,
                             name=f"xst{st}_{j}") for j in range(4)]
            src = xT[:, st * 512:(st + 1) * 512].rearrange(
                "(t p) s -> p t s", p=128)
            for j in range(4):
                nc.sync.dma_start(ts[j], src[:, j * 4:(j + 1) * 4, :])
            return x_slice_maker(ts)

        # st=0 x: split like wq so dt=0 lands first
        x0src = xT[:, 0:512].rearrange("(t p) s -> p t s", p=128)
        x0 = [xpool.tile([128, n, 512], bf16, tag=f"x0s{j}", bufs=1,
                         name=f"xst0_{j}") for j, n in enumerate(DT_SPLIT)]
        # issue the two tiny dt=0 DMAs first
        nc.sync.dma_start(wq_sb[0], wqr[:, 0:1, :])
        nc.sync.dma_start(x0[0], x0src[:, 0:1, :])
        for j in range(1, len(DT_SPLIT)):
            s0, n = DT_START[j], DT_SPLIT[j]
            nc.sync.dma_start(wq_sb[j], wqr[:, s0:s0 + n, :])
            nc.sync.dma_start(x0[j], x0src[:, s0:s0 + n, :])
        xtiles[0] = x0_slice_maker(x0)
        nc.sync.dma_start(wk_sb, wkT.rearrange("(t p) e -> p t e", p=128))
        nc.sync.dma_start(wv_sb, wvT.rearrange("(t p) e -> p t e", p=128))

        def wslice(kind, dt):
            if kind == "q":
                ti, off = _dt_loc(dt)
                return wq_sb[ti][:, off, :]
            return (wk_sb if kind == "k" else wv_sb)[:, dt, :]

        # --- constants ---
        cos_sb = const.tile([128, SS, DH], bf16)
        sin_sb = const.tile([128, SS, DH], bf16)
        nc.sync.dma_start(cos_sb, cosd.rearrange("p (b d) -> p b d", d=DH))
        nc.sync.dma_start(sin_sb, sind.rearrange("p (b d) -> p b d", d=DH))
        tri = const.tile([128, 128], bf16)
        nc.sync.dma_start(tri, trid[:, :])
        ind8 = const.tile([8, 512], bf16)
        nc.sync.dma_start(ind8, ind8d[:, :])
        nc.sync.dma_start(wo_sb, woT.rearrange("(t p) e -> p t e", p=128))
        identf = const.tile([128, 128], f32)
        make_identity(nc, identf)
        ident = const.tile([128, 128], bf16)
        make_identity(nc, ident)

        # --- persistent activations ---
        qTall = qkv.tile([128, ET, S], bf16, tag="qT")
        kTall = qkv.tile([128, ET, S], bf16, tag="kT")
        v_sb = qkv.tile([128, SS, HPC, 66], bf16, tag="v")
        # double-buffered stash: apply(i) overlaps attn(i+1)
        stash = [qkv.tile([65, HPC, 512], bf16, tag=f"stash{m}",
                          name=f"stash{m}")
                 for m in range(2)]
        nc.vector.memset(v_sb[:, :, :, 64:66], 1.0)

        def norm_rope(ps, dstT, st, su):
            """psum [si,e] natural -> per-head l2norm, rope, bf16,
            -> DMA-transpose into dstT columns."""
            sblk = st * 4 + su
            sq = work.tile([128, E], bf16, tag="sq", bufs=2)
            nc.scalar.square(sq, ps)
            ssq = work.tile([128, HPC], f32, tag="ssq", bufs=2)
            nc.vector.tensor_reduce(
                ssq, sq.rearrange("p (h d) -> p h d", d=DH),
                axis=AX.X, op=OP.add)
            # rsqrt via magic-number seed + 2 Newton iterations (DVE only)
            inv = work.tile([128, HPC], f32, tag="inv", bufs=2)
            ssq_i = ssq.bitcast(mybir.dt.int32)
            inv_i = inv.bitcast(mybir.dt.int32)
            nc.vector.tensor_scalar(inv_i, ssq_i, 1, None,
                                    op0=OP.arith_shift_right)
            nc.vector.tensor_scalar(inv_i, inv_i, 0x5f3759df, -1,
                                    op0=OP.subtract, op1=OP.mult)
            y2 = work.tile([128, HPC], f32, tag="y2", bufs=2)
            for _ in range(2):
                nc.vector.tensor_mul(y2, inv, inv)
                nc.vector.scalar_tensor_tensor(
                    y2, ssq, -0.5, y2, op0=OP.mult, op1=OP.mult)
                nc.vector.tensor_scalar(y2, y2, 1.5, None, op0=OP.add)
                nc.vector.tensor_mul(inv, inv, y2)
            qn = work.tile([128, HPC, DH], bf16, tag="qn", bufs=2)
            nc.vector.tensor_mul(
                qn, ps.rearrange("p (h d) -> p h d", d=DH),
                inv.unsqueeze(2).broadcast_to([128, HPC, DH]))
            cosb = cos_sb[:, sblk:sblk + 1, :].broadcast_to([128, HPC, DH])
            sinb = sin_sb[:, sblk:sblk + 1, :].broadcast_to([128, HPC, DH])
            rot = work.tile([128, HPC, 2, 32], bf16, tag="rot", bufs=2)
            qn4 = qn.rearrange("p h (t u) -> p h t u", u=32)
            nc.vector.tensor_copy(rot[:, :, 0:1, :], qn4[:, :, 1:2, :])
            nc.vector.tensor_copy(rot[:, :, 1:2, :], qn4[:, :, 0:1, :])
            nc.vector.tensor_mul(rot.rearrange("p h t u -> p h (t u)"),
                                 rot.rearrange("p h t u -> p h (t u)"), sinb)
            nc.vector.tensor_mul(qn, qn, cosb)
            qo = work.tile([128, E], bf16, tag="qo", bufs=2)
            nc.vector.tensor_add(
                qo, qn.rearrange("p h d -> p (h d)"),
                rot.rearrange("p h t u -> p (h t u)"))
            nc.sync.dma_start_transpose(
                dstT[:, :, sblk * 128:(sblk + 1) * 128], qo)

        def proj_half_gen(kind, st, s0, wide=False):
            """One su-pair of a proj wave: 4 chunks of 8 matmuls (yields
            between chunks so attention can interleave). wide=True uses
            psG+psV (4-bank rotation) for the pre-attention wave."""
            xt = xtiles[st]
            pools = (psG, psV) if wide else (psG, psG)
            tags = ("g", "pv") if wide else ("g", "g")
            prs = [pools[j].tile([128, E], f32, tag=tags[j],
                                 name=f"p{kind}{st}_{s0 + j}")
                   for j in range(2)]
            for dtc in range(4):
                for dt in range(dtc * 4, dtc * 4 + 4):
                    ws = wslice(kind, dt)
                    for j in range(2):
                        su = s0 + j
                        nc.tensor.matmul(
                            prs[j],
                            xt(dt)[:, su * 128:(su + 1) * 128],
                            ws,
                            start=(dt == 0), stop=(dt == DT - 1))
                if dtc < 3:
                    yield
            for j in range(2):
                su = s0 + j
                if kind == "v":
                    nc.vector.tensor_copy(
                        v_sb[:, st * 4 + su, :, 0:64],
                        prs[j].rearrange("p (h d) -> p h d", d=DH))
                else:
                    norm_rope(prs[j], qTall if kind == "q" else kTall,
                              st, su)

        def proj_half(kind, st, s0, wide=False):
            for _ in proj_half_gen(kind, st, s0, wide):
                pass

        def proj_all(st):
            for kind in ("q", "k", "v"):
                for s0 in (0, 2):
                    proj_half(kind, st, s0, wide=True)

        def attn_pair(i, ha, hb, feed):
            """Head-paired attention: heads (h, h+2) share PE tile config.
            Emission order per p: exp(p) -> lg4(p+1) -> feed -> pv(p)."""
            last = 4 * i + 3
            npr = 2 * (i + 1)
            hp = (ha % 2) * 64
            ets = {ha: ha // 2, hb: hb // 2}
            pvs = {h: psV.tile([128, 512], f32, tag="pv",
                               name=f"pv{i}_{h}")
                   for h in (ha, hb)}
            lgs = {}

            def lg4(p):
                for h in (ha, hb):
                    lgs[(h, p)] = psL.tile(
                        [128, 2, 512], f32, tag="lg",
                        name=f"lg{i}_{h}_{p}")
                # h-outer: head a's matmuls aren't queued behind head b's
                # psum-slot wait (slots free per-head as each exp finishes).
                # trim at pair granularity (not per-b): the extra 128 cols
                # of the upper diagonal plane are computed (garbage above
                # the diagonal) so exp can run as ONE call per (h,p); pv
                # trims them away.
                c0 = max(0, (2 * p - 4 * i)) * 128
                for h in (ha, hb):
                    for b in range(2):
                        sjb = 2 * p + b
                        nc.tensor.matmul(
                            lgs[(h, p)][:, b, c0:],
                            kTall[hp:hp + 64, ets[h],
                                  sjb * 128:(sjb + 1) * 128],
                            qTall[hp:hp + 64, ets[h],
                                  i * 512 + c0:(i + 1) * 512],
                            start=True, stop=True)

            lg4(0)
            for p in range(npr):
                c0p = max(0, (2 * p - 4 * i)) * 128
                exs = {}
                for h in (ha, hb):
                    lg2 = lgs.pop((h, p))
                    ex = expool.tile([128, 2, 512], bf16, tag="ex",
                                     name=f"ex{i}_{h}_{p}")
                    nc.scalar.activation(ex[:, :, c0p:],
                                         lg2[:, :, c0p:], AF.Exp)
                    exs[h] = ex
                if p + 1 < npr:
                    lg4(p + 1)
                feed()
                if i == 0:
                    feed()
                for b in range(2):
                    sjb = 2 * p + b
                    r = sjb - 4 * i
                    c0 = r * 128 if r > 0 else 0
                    if r >= 0:
                        for h in (ha, hb):
                            nc.gpsimd.tensor_mul(
                                exs[h][:, b, r * 128:(r + 1) * 128],
                                exs[h][:, b, r * 128:(r + 1) * 128],
                                tri)
                    for h in (ha, hb):
                        nc.tensor.matmul(
                            pvs[h][0:66, c0:],
                            v_sb[:, sjb, h, :],
                            exs[h][:, b, c0:],
                            start=(sjb == 0), stop=(sjb == last))
            # split across engines: a late stash copy delays the next
            # pair's psV slot (GpSimd can't read PSUM, so DVE + Act; Act
            # gets an even partition count — 65 came back corrupted)
            nc.vector.tensor_copy(stash[i % 2][:, ha, :], pvs[ha][0:65, :])
            nc.scalar.copy(stash[i % 2][0:64, hb, :], pvs[hb][0:64, :])
            nc.vector.tensor_copy(stash[i % 2][64:65, hb, :],
                                  pvs[hb][64:65, :])

        def norm_chain_gen(i):
            """den gather -> reciprocal -> per-et broadcast + apply, as a
            feedable generator. Reads stash[i%2], writes qTall cols of i."""
            sb = stash[i % 2]
            den = work.tile([8, 512], bf16, tag="den", bufs=2,
                            name=f"den{i}")
            nc.scalar.dma_start(den, sb[64:65, :, :])
            yield
            invT = psG.tile([128, 32], bf16, tag="g", name=f"invT{i}")
            for c in range(4):
                nc.tensor.transpose(
                    invT[:, c * 8:(c + 1) * 8],
                    den[:, c * 128:(c + 1) * 128], ident[0:8, 0:8])
            inv_sb = work.tile([128, 32], f32, tag="invsb", bufs=2,
                               name=f"invsb{i}")
            nc.vector.reciprocal(inv_sb, invT)
            yield
            invrow = psG.tile([8, 4, 128], f32, tag="g", name=f"invrow{i}")
            for c in range(4):
                nc.tensor.transpose(
                    invrow[:, c, :], inv_sb[:, c * 8:(c + 1) * 8], identf)
            inv_row = work.tile([8, 512], bf16, tag="invrowsb", bufs=2,
                                name=f"invrowsb{i}")
            nc.vector.tensor_copy(
                inv_row, invrow.rearrange("p c j -> p (c j)"))
            yield
            for et in range(ET):
                # [8 -> 128, 512]: partitions 0-63 = head 2et's 1/den,
                # 64-127 = head 2et+1's
                bc = psG.tile([128, 512], f32, tag="g", name=f"bc{i}_{et}")
                nc.tensor.matmul(bc, ind8[:, et * 128:(et + 1) * 128],
                                 inv_row, start=True, stop=True)
                for m in range(2):
                    h = 2 * et + m
                    hp = m * 64
                    nc.vector.tensor_mul(
                        qTall[hp:hp + 64, et, i * 512:(i + 1) * 512],
                        sb[0:64, h, :], bc[hp:hp + 64, :])
                yield

        def yproj_quarter_gen(ib, tail=False):
            pools = (psG, psV) if tail else (psG, psG)
            tags = ("g", "pv") if tail else ("g", "g")
            for nd0 in (0, 2):
                pss = [pools[j].tile([128, 512], f32, tag=tags[j],
                                     name=f"y{ib}_{nd0 + j}")
                       for j in range(2)]
                for ket in range(ET):
                    for j in range(2):
                        nd = nd0 + j
                        nc.tensor.matmul(
                            pss[j],
                            qTall[:, ket, ib * 128:(ib + 1) * 128],
                            wo_sb[:, ket, nd * 512:(nd + 1) * 512],
                            start=(ket == 0), stop=(ket == ET - 1))
                for j in range(2):
                    nd = nd0 + j
                    ys = work.tile([128, 512], bf16, tag="ys", bufs=4,
                                   name=f"ys{ib}_{nd}")
                    # Act only in the tail (it is exp-saturated during attn)
                    if tail and nd % 2 == 1:
                        nc.scalar.copy(ys, pss[j])
                    else:
                        nc.vector.tensor_copy(ys, pss[j])
                    nc.sync.dma_start(
                        Y[ib * 128:(ib + 1) * 128,
                          nd * 512:(nd + 1) * 512], ys)
                if nd0 == 0:
                    yield

        xtiles[1] = load_x(1)
        proj_all(0)
        yq_backlog = []
        pairs = ((0, 2), (4, 6), (1, 3), (5, 7))
        halves = (("q", 0), ("q", 2), ("k", 0), ("k", 2), ("v", 0), ("v", 2))
        for st in range(SB):
            nxt = st + 1 < SB
            if st + 2 < SB:
                xtiles[st + 2] = load_x(st + 2)
            gq = []
            if st > 0:
                gq.append(norm_chain_gen(st - 1))
            if nxt:
                gq += [proj_half_gen(k, st + 1, s) for (k, s) in halves]
            if st > 0:
                yq_backlog.extend(range(4 * (st - 1), 4 * st))
            # defer all yproj to st=3: supply≈demand per phase, and DVE is
            # idle there (no norm_rope) to absorb the ys copies
            ntake = len(yq_backlog) if not nxt else 0
            for _ in range(ntake):
                gq.append(yproj_quarter_gen(yq_backlog.pop(0)))

            def feed():
                while gq:
                    try:
                        next(gq[0])
                        return
                    except StopIteration:
                        gq.pop(0)

            for ha, hb in pairs:
                attn_pair(st, ha, hb, feed)
            while gq:
                feed()
        # tail: normalize block 3, then its 4 yproj quarters on 4 psum banks
        for g in [norm_chain_gen(SB - 1)] + [
                yproj_quarter_gen(ib, tail=True) for ib in range(12, 16)]:
            for _ in g:
                pass

    return nc


def _host_prep(x, wq, wk, wv, wo, qk_scale):
    """Returns per-core input dicts."""
    perm = np.concatenate([np.arange(0, DH, 2), np.arange(1, DH, 2)])
    wq_n = _l2n(wq, -1).reshape(HEADS, DH, DIM)[:, perm, :].reshape(HEADS * DH, DIM)
    wk_n = _l2n(wk, -1).reshape(HEADS, DH, DIM)[:, perm, :].reshape(HEADS * DH, DIM)
    wv_n = _l2n(wv, -1)
    wo_n = _l2n(wo, 0)
    sp = qk_scale.astype(np.float64)[perm]

    # rope tables with qk_scale folded in; permuted-block layout
    half = np.arange(0, DH, 2)
    freqs = 1.0 / (THETA ** (half.astype(np.float64) / DH))      # (32,)
    ang = np.arange(S, dtype=np.float64)[:, None] * freqs[None]  # (S, 32)
    cos_h, sin_h = np.cos(ang), np.sin(ang)
    cos_p = np.concatenate([cos_h, cos_h], 1)                    # (S, 64)
    sin_e = np.concatenate([-sin_h, sin_h], 1)
    cos_eff = (cos_p * sp[None, :]).astype(np.float32)
    swap_sp = np.concatenate([sp[32:], sp[:32]])
    sin_eff = (sin_e * swap_sp[None, :]).astype(np.float32)
    # device layout [128, SS*DH]: [p, b*64+c] = tbl[b*128+p, c]
    cosd = np.ascontiguousarray(
        cos_eff.reshape(SS, 128, DH).transpose(1, 0, 2).reshape(128, SS * DH))
    sind = np.ascontiguousarray(
        sin_eff.reshape(SS, 128, DH).transpose(1, 0, 2).reshape(128, SS * DH))

    # causal triangle for the diagonal 128-blocks: keep sjl <= sil
    sjl = np.arange(128)[:, None]
    sil = np.arange(128)[None, :]
    trid = (sjl <= sil).astype(np.float32)

    # indicator for denominator broadcast: ind8[k, h*64+m] = (k == h)
    ind8 = np.zeros((8, 512), dtype=np.float32)
    for h in range(8):
        ind8[h, h * 64:(h + 1) * 64] = 1.0

    in_maps = []
    for c in range(NCORES):
        b, t = divmod(c, TP)
        e0 = t * E
        in_maps.append({
            "xT": np.ascontiguousarray(x[b].T).astype(BF16),
            "wqT": np.ascontiguousarray(wq_n[e0:e0 + E].T).astype(BF16),
            "wkT": np.ascontiguousarray(wk_n[e0:e0 + E].T).astype(BF16),
            "wvT": np.ascontiguousarray(wv_n[e0:e0 + E].T).astype(BF16),
            "woT": np.ascontiguousarray(wo_n[:, e0:e0 + E].T).astype(BF16),
            "cosd": cosd.astype(BF16), "sind": sind.astype(BF16),
            "trid": trid.astype(BF16), "ind8d": ind8.astype(BF16),
        })
    return in_maps


def _install_profile_hook():
    """antenv.axon_hooks is absent in this image; shim it and register the
    ctypes NTFF hook against /opt/axon/libaxon_pjrt.so (mirrors trn_boot)."""
    import types
    import ctypes
    import contextlib

    try:
        from antenv.axon_hooks import get_axon_ntff_profile_hook  # noqa
        return
    except ImportError:
        pass
    import antenv
    mod = types.ModuleType("antenv.axon_hooks")
    state = {}
    mod.set_axon_ntff_profile_hook = lambda h: state.__setitem__("h", h)
    mod.get_axon_ntff_profile_hook = lambda: state.get("h")
    sys.modules["antenv.axon_hooks"] = mod
    antenv.axon_hooks = mod

    so_path = "/opt/axon/libaxon_pjrt.so"
    lib = ctypes.CDLL(so_path)
    if not hasattr(lib, "axon_start_nrt_profile"):
        return
    lib.axon_start_nrt_profile.argtypes = [
        ctypes.POINTER(ctypes.c_int64), ctypes.c_size_t]
    lib.axon_start_nrt_profile.restype = ctypes.c_int64
    lib.axon_stop_nrt_profile.argtypes = [ctypes.c_char_p]
    lib.axon_stop_nrt_profile.restype = ctypes.c_int64

    @contextlib.contextmanager
    def _hook(output_dir, device_ids):
        import jax
        jax.devices()
        if device_ids:
            ids = (ctypes.c_int64 * len(device_ids))(*device_ids)
            rc = lib.axon_start_nrt_profile(ids, len(device_ids))
        else:
            rc = lib.axon_start_nrt_profile(None, 0)
        if rc != 0:
            raise RuntimeError(f"axon_start_nrt_profile rc={rc}")
        try:
            yield
        finally:
            n = lib.axon_stop_nrt_profile(str(output_dir).encode())
            print(f"profile: {n} file(s) written to {output_dir}",
                  file=sys.stderr)

    mod.set_axon_ntff_profile_hook(_hook)


def kernel(x, wq, wk, wv, wo, qk_scale, _profile=False):
    from concourse.bass_utils import run_bass_kernel_spmd

    if _profile:
        _install_profile_hook()

    if "nc" not in _CACHE:
        nc = _build_program()
        nc.finalize()
        _CACHE["nc"] = nc
    nc = _CACHE["nc"]
    in_maps = _host_prep(np.asarray(x), np.asarray(wq), np.asarray(wk),
                         np.asarray(wv), np.asarray(wo), np.asarray(qk_scale))
    res = run_bass_kernel_spmd(nc, in_maps, core_ids=list(range(NCORES)),
                               trace=_profile)
    outs = res.results
    y = np.empty((B, S, DIM), dtype=np.float32)
    for b in range(B):
        y[b] = sum(np.asarray(outs[b * TP + t]["Y"], dtype=np.float32)
                   for t in range(TP))
    if _profile:
        _CACHE["last_exec_time_ns"] = res.exec_time_ns
        _CACHE["last_profile"] = res.profile_json
    return y
